# revision 1
# baseline (speedup 1.0000x reference)
"""GCN (5x GCNConv + global_mean_pool + 2-layer MLP) on 8 Trainium2 cores.

Strategy (per the node-partition sharding hint):
  - Nodes are sharded across 8 cores (12500 each, padded to 12800 = 25x512).
  - Per layer: each core GEMMs its shard (feat-major), scales by dinv,
    transposes to node-major, AllGathers the scaled features in 4 quarter
    chunks (25600 rows each -> int16-indexable), then gathers source rows
    per edge with dma_gather and scatter-adds into its destination shard
    via one-hot matmuls accumulated in PSUM (dinv post-scale folded into
    the one-hot matrices).
  - Degrees are computed on device with the same one-hot matmuls
    (ones-vector contraction), dinv = sqrt(1/max(deg, 0.5)).
  - Per-graph mean pooling via one-hot (node->graph) matmuls per layer;
    partial sums + counts AllReduced, then the small MLP runs replicated
    on every core; core 0's output is returned.
All compute dtypes: fp16 storage / fp32 accumulation (PSUM), MLP in fp32.
"""

import numpy as np
import ml_dtypes

NC = 8
_G_DEFAULT = 512
FP16 = np.float16


def _ceil_to(a, m):
    return -(-a // m) * m


def _preprocess(x, edge_index, batch, n_graphs):
    """Build per-core edge streams and static structure."""
    N, D = x.shape
    assert N % NC == 0
    SH = N // NC                      # real rows per core
    SHP = _ceil_to(SH, 512)           # padded rows per core
    QT = SHP // 4                     # quarter (AllGather chunk per core)
    NT = SHP // 128                   # node tiles per core
    NST = SHP // 512                  # supertiles per core
    TBL = NC * QT                     # rows per gathered chunk table
    assert TBL < 32768, "int16 gather index overflow"
    G = n_graphs

    row = np.asarray(edge_index[0], dtype=np.int64)
    col = np.asarray(edge_index[1], dtype=np.int64)
    # self-loops are NOT materialized as edges; their contribution is added
    # during PSUM evacuation (dinv*u term) and deg gets +1 on device.

    kd = col // SH                    # destination core
    ld = col - kd * SH                # local dst row
    ks = row // SH                    # source core
    rr = row - ks * SH
    jq = rr // QT                     # source quarter (0..3)
    idx16 = (ks * QT + (rr - jq * QT)).astype(np.int64)
    tile = ld // 128
    stile = tile // 4

    # per-core sorted streams
    per_core = []
    for k in range(NC):
        m = kd == k
        o = np.lexsort((ld[m], tile[m], jq[m], stile[m]))
        per_core.append({
            "tile": tile[m][o], "j": jq[m][o],
            "idx16": idx16[m][o], "ld": ld[m][o],
        })

    # static padded cell sizes: cell = (tile, j); S multiple of 16,
    # cross-core max; force >=16 for j==0 so every tile gets at least one
    # scatter-matmul window (writes its psum region, incl. padding tiles).
    ncell = NT * 4
    S = np.zeros(ncell, dtype=np.int64)
    for k in range(NC):
        ck = per_core[k]["tile"] * 4 + per_core[k]["j"]
        cnt = np.bincount(ck, minlength=ncell)
        S = np.maximum(S, cnt)
    S = _ceil_to(S, 16)
    S[0::4] = np.maximum(S[0::4], 16)

    # cell order: (st, j, tile): for gather groups (st, j) contiguous
    cell_order = []
    for st in range(NST):
        for j in range(4):
            for a in range(4):
                cell_order.append((4 * st + a) * 4 + j)
    cell_order = np.array(cell_order)
    # slot layout: groups (st, j) are padded to 128-multiples; cells within
    # a group are 16-granular and contiguous.
    cell_off = np.zeros(ncell, dtype=np.int64)   # slot offset by cell id
    off = 0
    for st in range(NST):
        for j in range(4):
            for a in range(4):
                c = (4 * st + a) * 4 + j
                cell_off[c] = off
                off += S[c]
            # pad the group end to 128
            off = _ceil_to(off, 128)
    TOT = off                                     # total slots per core
    NCHUNK = TOT // 128

    # group (st, j) sizes/offsets for gathers; gather num_idxs is the
    # group size rounded up to 128 so every token-tile slot gets written
    # (pad idxs 0, pad scal -1000).
    groups = []      # (st, j, slot_off, slots, padded_slots)
    gpad_total = 0
    for st in range(NST):
        for j in range(4):
            c0 = (4 * st) * 4 + j
            goff = cell_off[c0]
            gsz = int(sum(S[(4 * st + a) * 4 + j] for a in range(4)))
            gpad = _ceil_to(gsz, 128)
            groups.append((st, j, int(goff), gsz, gpad))
            gpad_total += gpad

    # fill per-core padded streams
    idx_slots = np.zeros((NC, TOT), dtype=np.int16)
    scal_slots = np.full((NC, TOT), -1000.0, dtype=np.float32)
    for k in range(NC):
        pk = per_core[k]
        ck = pk["tile"] * 4 + pk["j"]
        # edges are sorted so each cell is one contiguous run; position
        # within the run + the cell's padded offset gives the slot.
        arange = np.arange(len(ck))
        if len(ck):
            starts_pos = np.concatenate(
                [[0], np.flatnonzero(np.diff(ck) != 0) + 1])
            first_occ = np.zeros(ncell, dtype=np.int64)
            first_occ[ck[starts_pos]] = starts_pos
            within = arange - first_occ[ck]
        else:
            within = arange
        slot = cell_off[ck] + within
        idx_slots[k, slot] = pk["idx16"].astype(np.int16)
        scal_slots[k, slot] = pk["ld"].astype(np.float32)

    # wrapped idx layout per gather group: [16, S/16] tiled to [128, S/16]
    IDXCOLS = TOT // 16
    idx_stream = np.zeros((NC, 128, IDXCOLS), dtype=np.int16)
    gcol_off = {}
    coff = 0
    for (st, j, goff, gsz, gpad) in groups:
        gcol_off[(st, j)] = coff
        if gpad == 0:
            continue
        blk = idx_slots[:, goff:goff + gpad].reshape(NC, gpad // 16, 16)
        blk = np.transpose(blk, (0, 2, 1))        # [NC, 16, S/16]
        idx_stream[:, :, coff:coff + gpad // 16] = np.tile(blk, (1, 8, 1))
        coff += gpad // 16

    # windows: one scatter-matmul per (128-slot chunk x intersecting cell).
    # scal column w holds ld - 128*tile_w for the chunk's 128 slots (slots of
    # other tiles fall outside [0,128) and never match; pads are -1000).
    ld_slots = scal_slots                     # [NC, TOT] raw ld (or -1000)
    win_cols = []                             # per-window [NC, 128] columns
    chunks = []
    for (st, j, goff, gsz, gpad) in groups:
        for a in range(4):
            c = (4 * st + a) * 4 + j
            if S[c] == 0:
                continue
            c0, c1 = cell_off[c], cell_off[c] + int(S[c])
            ch_lo, ch_hi = c0 // 128, (c1 - 1) // 128
            for ci in range(ch_lo, ch_hi + 1):
                slot0 = ci * 128
                colv = ld_slots[:, slot0:slot0 + 128] - 128.0 * (4 * st + a)
                win_cols.append(colv.astype(np.float32))
                chunks.append(dict(
                    st=st, j=j, a=a,
                    tok_col=int(slot0 - goff) // 128,
                    scal_col=len(win_cols) - 1,
                    base=a * 128,
                ))
    NCHUNK = len(win_cols)
    scal_stream = np.stack(win_cols, axis=2)  # [NC, 128, NWIN]

    # pooling: batch scalars per core per node tile [128, NT]
    batch = np.asarray(batch, dtype=np.int64)
    batch_scal = np.full((NC, 128, NT), -1000.0, dtype=np.float32)
    for k in range(NC):
        bs = batch[k * SH:(k + 1) * SH].astype(np.float32)
        pad = np.full(SHP - SH, -1000.0, dtype=np.float32)
        bp = np.concatenate([bs, pad]).reshape(NT, 128).T
        batch_scal[k] = bp

    # AG-in DMA segments per supertile: list of (tile_a0, ntiles, j, rowoff)
    ag_segs = []
    for st in range(NST):
        segs = []
        a = 0
        while a < 4:
            base = 512 * st + 128 * a
            j = base // QT
            r = base - j * QT
            n = 1
            while a + n < 4 and (base + 128 * n) // QT == j:
                n += 1
            segs.append((a, n, j, r))
            a += n
        ag_segs.append(segs)

    meta = dict(
        N=N, D=D, SH=SH, SHP=SHP, QT=QT, NT=NT, NST=NST, TBL=TBL, G=G,
        TOT=TOT, NCHUNK=NCHUNK, IDXCOLS=IDXCOLS,
        groups=groups, gcol_off=gcol_off, chunks=chunks, ag_segs=ag_segs,
        idx_stream=idx_stream, scal_stream=scal_stream, batch_scal=batch_scal,
    )
    return meta


def _build(meta):
    """Construct the Bass module (SPMD; identical program on 8 cores)."""
    import os
    import concourse.mybir as mybir
    import concourse.bacc as bacc
    import concourse.tile as tile

    STAGE = int(os.environ.get("KSTAGE", "99"))
    NLAYER = int(os.environ.get("KLAYERS", "5"))

    f32 = mybir.dt.float32
    fp16 = mybir.dt.float16
    i16 = mybir.dt.int16

    SHP, QT, NT, NST, TBL, G = (meta["SHP"], meta["QT"], meta["NT"],
                                meta["NST"], meta["TBL"], meta["G"])
    NCHUNK, IDXCOLS = meta["NCHUNK"], meta["IDXCOLS"]
    groups, gcol_off, chunks, ag_segs = (meta["groups"], meta["gcol_off"],
                                         meta["chunks"], meta["ag_segs"])
    MAXGCOL = max((g[4] // 128 for g in groups), default=1)

    nc = bacc.Bacc("TRN2", target_bir_lowering=False, debug=False,
                   enable_asserts=False, num_devices=NC)

    # ---- I/O ----
    xT_in = nc.dram_tensor("xT_in", [128, SHP], f32, kind="ExternalInput")
    idx_in = nc.dram_tensor("idx_in", [128, IDXCOLS], i16, kind="ExternalInput")
    scal_in = nc.dram_tensor("scal_in", [128, NCHUNK], f32, kind="ExternalInput")
    bscal_in = nc.dram_tensor("bscal_in", [128, NT], f32, kind="ExternalInput")
    w_in = nc.dram_tensor("w_in", [5 * 128, 128], fp16, kind="ExternalInput")
    ball_in = nc.dram_tensor("ball_in", [128, 5], f32, kind="ExternalInput")
    iota128_in = nc.dram_tensor("iota128_in", [128, 128], fp16, kind="ExternalInput")
    iotag_in = nc.dram_tensor("iotag_in", [128, G], fp16, kind="ExternalInput")
    onesc_in = nc.dram_tensor("onesc_in", [128, 1], fp16, kind="ExternalInput")
    onesr_in = nc.dram_tensor("onesr_in", [1, 128], f32, kind="ExternalInput")
    ident_in = nc.dram_tensor("ident_in", [128, 128], fp16, kind="ExternalInput")
    wl1_in = nc.dram_tensor("wl1_in", [640, 640], f32, kind="ExternalInput")
    bl1_in = nc.dram_tensor("bl1_in", [128, 5], f32, kind="ExternalInput")
    wl2_in = nc.dram_tensor("wl2_in", [128, 5], f32, kind="ExternalInput")
    bl2_in = nc.dram_tensor("bl2_in", [1, 1], f32, kind="ExternalInput")
    out_ext = nc.dram_tensor("out", [G], f32, kind="ExternalOutput")

    # ---- internal DRAM (collectives) ----
    ag_ins, ag_outs = [], []
    for l in range(5):
        ag_ins.append([nc.dram_tensor(f"agi_{l}_{j}", [QT, 128], fp16,
                                      kind="Internal") for j in range(4)])
        ag_outs.append([nc.dram_tensor(f"ago_{l}_{j}", [TBL, 128], fp16,
                                       kind="Internal", addr_space="Shared")
                        for j in range(4)])
    ar_in = nc.dram_tensor("ar_in", [641, 512], f32, kind="Internal")
    ar_out = nc.dram_tensor("ar_out", [641, 512], f32, kind="Internal",
                            addr_space="Shared")

    AOT = mybir.AluOpType
    AFT = mybir.ActivationFunctionType

    with tile.TileContext(nc) as tc:
        with tc.tile_pool(name="const", bufs=1) as cpool, \
             tc.tile_pool(name="stream", bufs=1) as spool, \
             tc.tile_pool(name="big", bufs=1) as bpool, \
             tc.tile_pool(name="work", bufs=2) as wpool, \
             tc.tile_pool(name="tokp", bufs=2) as tokpool, \
             tc.tile_pool(name="mp", bufs=4) as mpool, \
             tc.tile_pool(name="psA", bufs=2, space="PSUM") as psA, \
             tc.tile_pool(name="psB", bufs=2, space="PSUM") as psB, \
             tc.tile_pool(name="psP", bufs=1, space="PSUM") as psP:

            # ---- constants ----
            iota128 = cpool.tile([128, 128], fp16)
            nc.sync.dma_start(iota128[:], iota128_in.ap())
            iotag = cpool.tile([128, G], fp16)
            nc.sync.dma_start(iotag[:], iotag_in.ap())
            onesc = cpool.tile([128, 1], fp16)
            nc.sync.dma_start(onesc[:], onesc_in.ap())
            onesr = cpool.tile([1, 128], f32)
            nc.sync.dma_start(onesr[:], onesr_in.ap())
            ident = cpool.tile([128, 128], fp16)
            nc.sync.dma_start(ident[:], ident_in.ap())
            w_sb = cpool.tile([128, 5, 128], fp16)
            nc.sync.dma_start(w_sb[:], w_in.ap().rearrange("(a p) b -> p a b", p=128))
            ball = cpool.tile([128, 5], f32)
            nc.sync.dma_start(ball[:], ball_in.ap())
            bscal = cpool.tile([128, NT], f32)
            nc.sync.dma_start(bscal[:], bscal_in.ap())

            idx_sb = spool.tile([128, IDXCOLS], i16)
            nc.sync.dma_start(idx_sb[:], idx_in.ap())
            scal_sb = spool.tile([128, NCHUNK], f32)
            nc.sync.dma_start(scal_sb[:], scal_in.ap())

            # y ping-pong buffers (feat-major, fp16)
            yT = [bpool.tile([128, SHP], fp16, name=f"yT{i}", tag=f"yT{i}")
                  for i in range(2)]
            nc.gpsimd.dma_start(yT[0][:], xT_in.ap())   # cast f32->fp16

            dinv_rep = bpool.tile([128, SHP], fp16)

            # ---- deg pass ----
            run_deg = STAGE >= 2
            run_gemm = STAGE >= 3
            run_spmm = STAGE >= 4
            run_pool = STAGE >= 5
            run_mlp = STAGE >= 6
            ch_by_st = {}
            for ch in chunks:
                ch_by_st.setdefault(ch["st"], []).append(ch)
            scope_deg = nc.named_scope("deg")
            scope_deg.__enter__()
            for st in range(NST if run_deg else 0):
                dps = psP.tile([1, 512], f32, tag="dps")
                cl = ch_by_st[st]
                for i, ch in enumerate(cl):
                    m = mpool.tile([128, 128], fp16, tag="M")
                    nc.vector.tensor_scalar(
                        m[:], iota128[:],
                        scal_sb[:, ch["scal_col"]:ch["scal_col"] + 1], None,
                        AOT.is_equal)
                    nc.tensor.matmul(dps[:, ch["base"]:ch["base"] + 128],
                                     onesc[:], m[:],
                                     start=(i == 0), stop=(i == len(cl) - 1))
                # dinv = sqrt(1/max(deg, 0.5)) for this supertile
                drow = wpool.tile([1, 512], f32, tag="drow")
                nc.vector.tensor_scalar(drow[:], dps[:], 1.0, 0.5, AOT.add, AOT.max)
                nc.vector.reciprocal(drow[:], drow[:])
                nc.scalar.activation(drow[:], drow[:], AFT.Sqrt)
                drps = psA.tile([128, 512], f32, tag="mm")
                nc.tensor.matmul(drps[:], onesr[:], drow[:],
                                 start=True, stop=True)
                nc.vector.tensor_copy(dinv_rep[:, 512 * st:512 * st + 512],
                                      drps[:])

            scope_deg.__exit__(None, None, None)
            pool_ps = None
            cnt_ps = None
            for l in range(NLAYER if run_gemm else 0):
                ycur, ynext = yT[l % 2], yT[(l + 1) % 2]
                # ---- GEMM + dinv + transpose -> AG inputs ----
                scope_g = nc.named_scope(f"L{l}_gemm")
                scope_g.__enter__()
                for st in range(NST):
                    s0 = 512 * st
                    ups = psA.tile([128, 512], f32, tag="mm")
                    nc.tensor.matmul(ups[:], w_sb[:, l, :],
                                     ycur[:, s0:s0 + 512], start=True, stop=True)
                    ut = ycur[:, s0:s0 + 512]   # reuse consumed input buffer
                    nc.vector.tensor_tensor(ut, ups[:],
                                            dinv_rep[:, s0:s0 + 512], AOT.mult)
                    trp = psB.tile([128, 512], fp16, tag="tr")
                    for a in range(4):
                        nc.tensor.transpose(trp[:, 128 * a:128 * a + 128],
                                            ut[:, 128 * a:128 * a + 128], ident[:])
                    agst = wpool.tile([128, 4, 128], fp16, tag="agst")
                    nc.vector.tensor_copy(
                        agst[:].rearrange("p a b -> p (a b)"), trp[:])
                    for (a0, ntil, j, roff) in ag_segs[st]:
                        nc.sync.dma_start(
                            ag_ins[l][j].ap()[roff:roff + 128 * ntil, :]
                            .rearrange("(a p) b -> p a b", p=128),
                            agst[:, a0:a0 + ntil, :])
                scope_g.__exit__(None, None, None)
                for j in range(4):
                    nc.gpsimd.collective_compute(
                        "AllGather", AOT.bypass,
                        replica_groups=[list(range(NC))],
                        ins=[ag_ins[l][j].ap().opt()],
                        outs=[ag_outs[l][j].ap().opt()])

                # ---- SpMM (gather + one-hot matmul scatter) ----
                scope_s = nc.named_scope(f"L{l}_spmm")
                scope_s.__enter__()
                for st in range(NST if run_spmm else 0):
                    s0 = 512 * st
                    zps = psA.tile([128, 512], f32, tag="mm")
                    toks = {}
                    for (gst, j, goff, gsz, gpad) in groups:
                        if gst != st or gpad == 0:
                            continue
                        tok = tokpool.tile([128, MAXGCOL, 128], fp16, tag="tok")
                        co = gcol_off[(st, j)]
                        nc.gpsimd.dma_gather(
                            tok[:, :gpad // 128, :], ag_outs[l][j].ap(),
                            idx_sb[:, co:co + gpad // 16],
                            num_idxs=gpad, num_idxs_reg=gpad, elem_size=128,
                            single_packet=False)
                        toks[j] = tok
                    cl = ch_by_st[st]
                    for i, ch in enumerate(cl):
                        m = mpool.tile([128, 128], fp16, tag="M")
                        d0 = s0 + ch["base"]
                        nc.vector.scalar_tensor_tensor(
                            m[:], iota128[:],
                            scal_sb[:, ch["scal_col"]:ch["scal_col"] + 1],
                            dinv_rep[:, d0:d0 + 128],
                            AOT.is_equal, AOT.mult)
                        nc.tensor.matmul(
                            zps[:, ch["base"]:ch["base"] + 128],
                            toks[ch["j"]][:, ch["tok_col"], :], m[:],
                            start=(i == 0), stop=(i == len(cl) - 1))
                    # y = relu(z + dinv*uT~ (self loop) + b)
                    selft = wpool.tile([128, 512], f32, tag="selft")
                    nc.vector.tensor_tensor(selft[:], ycur[:, s0:s0 + 512],
                                            dinv_rep[:, s0:s0 + 512], AOT.mult)
                    nc.vector.tensor_tensor(selft[:], zps[:], selft[:], AOT.add)
                    nc.scalar.activation(ynext[:, s0:s0 + 512], selft[:],
                                         AFT.Relu, bias=ball[:, l:l + 1])
                    # ---- pooling of ynext (node->graph one-hot matmul) ----
                    if not run_pool:
                        continue
                    trp2 = psB.tile([128, 512], fp16, tag="tr")
                    for a in range(4):
                        nc.tensor.transpose(trp2[:, 128 * a:128 * a + 128],
                                            ynext[:, s0 + 128 * a:s0 + 128 * (a + 1)],
                                            ident[:])
                    ynm = wpool.tile([128, 4, 128], fp16, tag="ynm")
                    nc.vector.tensor_copy(
                        ynm[:].rearrange("p a b -> p (a b)"), trp2[:])
                    if st == 0:
                        pool_ps = psP.tile([128, 512], f32, tag="pool")
                        if l == 0:
                            cnt_ps = psP.tile([1, 512], f32, tag="cnt")
                    for a in range(4):
                        t = 4 * st + a
                        mp = mpool.tile([128, G], fp16, tag="Mp")
                        nc.vector.tensor_scalar(
                            mp[:], iotag[:], bscal[:, t:t + 1], None,
                            AOT.is_equal)
                        first = (st == 0 and a == 0)
                        last = (st == NST - 1 and a == 3)
                        nc.tensor.matmul(pool_ps[:, :G], ynm[:, a, :], mp[:],
                                         start=first, stop=last)
                        if l == 0:
                            nc.tensor.matmul(cnt_ps[:, :G], onesc[:], mp[:],
                                             start=first, stop=last)
                scope_s.__exit__(None, None, None)
                if not run_pool:
                    continue
                arst = wpool.tile([128, 512], f32, tag="arst")
                nc.vector.tensor_copy(arst[:, :G], pool_ps[:, :G])
                if G < 512:
                    nc.vector.memset(arst[:, G:], 0.0)
                nc.sync.dma_start(ar_in.ap()[128 * l:128 * (l + 1), :], arst[:])
                if l == 0:
                    cst = wpool.tile([1, 512], f32, tag="cst")
                    nc.vector.tensor_copy(cst[:, :G], cnt_ps[:, :G])
                    if G < 512:
                        nc.vector.memset(cst[:, G:], 0.0)
                    nc.sync.dma_start(ar_in.ap()[640:641, :], cst[:])

            if not run_mlp:
                dummy = wpool.tile([1, 512], f32, tag="dumo")
                nc.vector.memset(dummy[:], 0.0)
                nc.sync.dma_start(
                    out_ext.ap().rearrange("(a b) -> a b", a=1), dummy[:, :G])
            if run_mlp:
                nc.gpsimd.collective_compute(
                    "AllReduce", AOT.add, replica_groups=[list(range(NC))],
                    ins=[ar_in.ap().opt()], outs=[ar_out.ap().opt()])

                # ---- MLP (replicated, fp32) ----
                wl1 = bpool.tile([128, 5, 640], f32)
                nc.sync.dma_start(wl1[:],
                                  wl1_in.ap().rearrange("(a p) b -> p a b", p=128))
                wl2 = cpool.tile([128, 5], f32)
                nc.sync.dma_start(wl2[:], wl2_in.ap())
                bl1 = cpool.tile([128, 5], f32)
                nc.sync.dma_start(bl1[:], bl1_in.ap())
                bl2 = cpool.tile([1, 1], f32)
                nc.sync.dma_start(bl2[:], bl2_in.ap())

                cnt = cpool.tile([1, 512], f32)
                nc.sync.dma_start(cnt[:], ar_out.ap()[640:641, :])
                nc.vector.tensor_scalar(cnt[:], cnt[:], 1.0, None, AOT.max)
                recip = cpool.tile([1, 512], f32)
                nc.vector.reciprocal(recip[:], cnt[:])
                rps = psA.tile([128, 512], f32, tag="mm")
                nc.tensor.matmul(rps[:], onesr[:], recip[:], start=True, stop=True)
                rrep = wpool.tile([128, 512], f32, tag="rrep")
                nc.vector.tensor_copy(rrep[:], rps[:])

                pm = [wpool.tile([128, 512], f32, tag=f"pm{t}", bufs=1,
                                 name=f"pm{t}") for t in range(5)]
                for t in range(5):
                    pt = wpool.tile([128, 512], f32, tag="pt")
                    nc.sync.dma_start(pt[:], ar_out.ap()[128 * t:128 * (t + 1), :])
                    nc.vector.tensor_tensor(pm[t][:], pt[:], rrep[:], AOT.mult)
                hs = [wpool.tile([128, 512], f32, tag=f"h{o}", bufs=1,
                                 name=f"h{o}") for o in range(5)]
                for o in range(5):
                    hps = psA.tile([128, 512], f32, tag="mm")
                    for i in range(5):
                        nc.tensor.matmul(hps[:], wl1[:, i, 128 * o:128 * (o + 1)],
                                         pm[i][:], start=(i == 0), stop=(i == 4))
                    nc.scalar.activation(hs[o][:], hps[:], AFT.Relu,
                                         bias=bl1[:, o:o + 1])
                yps = psP.tile([1, 512], f32, tag="yf")
                for i in range(5):
                    nc.tensor.matmul(yps[:], wl2[:, i:i + 1], hs[i][:],
                                     start=(i == 0), stop=(i == 4))
                ysb = wpool.tile([1, 512], f32, tag="ysb")
                nc.scalar.activation(ysb[:], yps[:], AFT.Identity, bias=bl2[:, 0:1])
                nc.sync.dma_start(out_ext.ap().rearrange("(a b) -> a b", a=1),
                                  ysb[:, :G])

    nc.compile()
    return nc


def _make_in_maps(meta, x, W_list, b_list, Wl1, bl1, Wl2, bl2):
    N, D, SH, SHP, NT, G = (meta["N"], meta["D"], meta["SH"], meta["SHP"],
                            meta["NT"], meta["G"])
    iota128 = np.tile(np.arange(128), (128, 1)).astype(FP16)
    iotag = np.tile(np.arange(G), (128, 1)).astype(FP16)
    onesc = np.ones((128, 1), FP16)
    onesr = np.ones((1, 128), np.float32)
    ident = np.eye(128).astype(FP16)
    w_stack = np.concatenate([w.astype(FP16) for w in W_list], axis=0)  # [640,128]
    ball = np.stack([b.astype(np.float32) for b in b_list], axis=1)     # [128,5]
    bl1m = np.asarray(bl1, np.float32).reshape(5, 128).T                # [128,5]
    wl2m = np.asarray(Wl2, np.float32).reshape(5, 128).T                # [128,5]
    wl1m = np.asarray(Wl1, np.float32)
    bl2m = np.asarray(bl2, np.float32).reshape(1, 1)

    in_maps = []
    for k in range(NC):
        xs = np.asarray(x[k * SH:(k + 1) * SH], np.float32)
        xT = np.zeros((128, SHP), np.float32)
        xT[:, :SH] = xs.T
        in_maps.append(dict(
            xT_in=xT, idx_in=meta["idx_stream"][k],
            scal_in=meta["scal_stream"][k], bscal_in=meta["batch_scal"][k],
            w_in=w_stack, ball_in=ball, iota128_in=iota128, iotag_in=iotag,
            onesc_in=onesc, onesr_in=onesr, ident_in=ident,
            wl1_in=wl1m, bl1_in=bl1m, wl2_in=wl2m, bl2_in=bl2m,
        ))
    return in_maps


_LAST_RESULT = {}


def kernel(x, edge_index, batch, W1, b1, W2, b2, W3, b3, W4, b4,
           Wl1, bl1, Wl2, bl2, n_graphs=_G_DEFAULT, trace=False):
    from concourse import bass_utils

    x = np.asarray(x)
    meta = _preprocess(x, np.asarray(edge_index), np.asarray(batch), n_graphs)
    nc = _build(meta)
    in_maps = _make_in_maps(meta, x, [W1, W2, W3, W4, W4],
                            [b1, b2, b3, b4, b4], Wl1, bl1, Wl2, bl2)
    res = bass_utils.run_bass_kernel_spmd(
        nc, in_maps, core_ids=list(range(NC)), trace=trace)
    _LAST_RESULT["res"] = res
    return res.results[0]["out"].astype(np.float32)



# revision 4
# speedup vs baseline: 2.3172x; 2.3172x over previous
"""GCN (5x GCNConv + global_mean_pool + 2-layer MLP) on 8 Trainium2 cores.

v2 design (node-partitioned, pull-based SpMM):
  - Nodes sharded 8 ways (12500/core, padded to 12800). Per layer:
    GEMM (feat-major) -> transpose -> AllGather of the [12800,128] fp16
    shard per quarter (4 chunks, int16-indexable 25600-row tables),
    fired eagerly as soon as each quarter's GEMM tiles are written.
  - SpMM: per (supertile, quarter) group, a 4-queue-rotated dma_gather
    pulls the group's edge-source rows (self-triggered SWDGE; rotating
    queue_num engages all 4 Q7 pairs and quadruples gather throughput).
    One-hot scatter matrices M = (iota==ld)*norm are built BATCHED on
    the Vector engine with stride-0 broadcast APs (2 ops per group) and
    consumed by per-chunk merged matmuls accumulating z in PSUM.
  - Graph norms (sym-normalized adjacency incl. self-loop weights) are
    computed on the HOST and folded into M / the self-term, so there is
    no on-device degree pass.
  - Self-loops are applied as z += dinv^2 * u via a vector multiply
    (dinv^2 replicated across partitions once at start).
  - Mean-pool one-hots (0/1) are built batched per supertile; per-graph
    1/count is folded in after the AllReduce (host-provided recip).
All compute fp16 storage / fp32 PSUM accumulation; MLP fp32.
"""

import numpy as np

NC = 8
_G_DEFAULT = 512
FP16 = np.float16


def _ceil_to(a, m):
    return -(-a // m) * m


def _preprocess(x, edge_index, batch, n_graphs):
    N, D = x.shape
    assert N % NC == 0
    SH = N // NC
    SHP = _ceil_to(SH, 512)
    QT = SHP // 4
    NT = SHP // 128
    NST = SHP // 512
    TBL = NC * QT
    assert TBL < 32768
    G = n_graphs

    row = np.asarray(edge_index[0], dtype=np.int64)
    col = np.asarray(edge_index[1], dtype=np.int64)

    # host-side GCN norm: deg = in-degree + 1 (self loop), dinv = deg^-1/2
    deg = np.bincount(col, minlength=N).astype(np.float64) + 1.0
    dinv = 1.0 / np.sqrt(deg)
    enorm = (dinv[row] * dinv[col]).astype(np.float32)
    dinvsq = (dinv * dinv).astype(np.float32)

    kd = col // SH
    ld = col - kd * SH
    ks = row // SH
    rr = row - ks * SH
    jq = rr // QT
    idx16 = (ks * QT + (rr - jq * QT)).astype(np.int64)
    tile = ld // 128

    per_core = []
    for k in range(NC):
        m = kd == k
        o = np.lexsort((ld[m], tile[m]))
        per_core.append({
            "tile": tile[m][o], "j": jq[m][o],
            "idx16": idx16[m][o], "ld": ld[m][o], "w": enorm[m][o],
        })

    # cells = (tile, j), padded to 16, cross-core max; >=16 for j=0 cells
    # so every supertile has at least one window (start=True coverage).
    ncell = NT * 4
    S = np.zeros(ncell, dtype=np.int64)
    for k in range(NC):
        ck = per_core[k]["tile"] * 4 + per_core[k]["j"]
        cnt = np.bincount(ck, minlength=ncell)
        S = np.maximum(S, cnt)
    S = _ceil_to(S, 16)
    S[0::4] = np.maximum(S[0::4], 16)

    # layout: groups (st, j); within a group the 4 cells (tiles a=0..3 of
    # the supertile) are contiguous; group padded to 128.
    cell_off = np.zeros(ncell, dtype=np.int64)
    groups = []            # (st, j, slot_off, padded_slots)
    off = 0
    for st in range(NST):
        for j in range(4):
            goff = off
            for a in range(4):
                c = (4 * st + a) * 4 + j
                cell_off[c] = off
                off += S[c]
            off = _ceil_to(off, 128)
            groups.append((st, j, int(goff), int(off - goff)))
    TOT = off

    # fill slot streams
    idx_slots = np.zeros((NC, TOT), dtype=np.int16)
    ld_slots = np.full((NC, TOT), -1000.0, dtype=np.float32)
    w_slots = np.zeros((NC, TOT), dtype=np.float32)
    for k in range(NC):
        pk = per_core[k]
        ck = pk["tile"] * 4 + pk["j"]
        arange = np.arange(len(ck))
        if len(ck):
            order = np.argsort(ck, kind="stable")
            ck_s = ck[order]
            starts_pos = np.concatenate([[0], np.flatnonzero(np.diff(ck_s) != 0) + 1])
            first_occ = np.full(ncell, -1, dtype=np.int64)
            first_occ[ck_s[starts_pos]] = starts_pos
            within = np.empty(len(ck), dtype=np.int64)
            within[order] = arange - first_occ[ck_s]
        else:
            within = arange
        slot = cell_off[ck] + within
        idx_slots[k, slot] = pk["idx16"].astype(np.int16)
        ld_slots[k, slot] = pk["ld"].astype(np.float32)
        w_slots[k, slot] = pk["w"]

    # idx wrap per group: [16, gpad/16] tiled to 128 partitions
    IDXCOLS = TOT // 16
    idx_stream = np.zeros((NC, 128, IDXCOLS), dtype=np.int16)
    gcol_off = {}
    coff = 0
    for (st, j, goff, gpad) in groups:
        gcol_off[(st, j)] = coff
        if gpad == 0:
            continue
        blk = idx_slots[:, goff:goff + gpad].reshape(NC, gpad // 16, 16)
        blk = np.transpose(blk, (0, 2, 1))
        idx_stream[:, :, coff:coff + gpad // 16] = np.tile(blk, (1, 8, 1))
        coff += gpad // 16

    # windows: one merged matmul per 128-slot chunk; one M column block per
    # (chunk x intersecting cell).  Blocks of a chunk are adjacent cells.
    ld_cols = []    # per block: [NC, 128] ld - 128*tile
    w_cols = []     # per block: [NC, 128] norm (0 for pads)
    win_by_group = {}   # (st,j) -> list of (tok_col, a_lo, nblk, mcol0)
    for (st, j, goff, gpad) in groups:
        wins = []
        for ci in range(gpad // 128):
            slot0 = goff + ci * 128
            # cells intersecting [slot0, slot0+128)
            blks = []
            for a in range(4):
                c = (4 * st + a) * 4 + j
                c0, c1 = cell_off[c], cell_off[c] + int(S[c])
                if c0 < slot0 + 128 and c1 > slot0 and S[c] > 0:
                    blks.append(a)
            if not blks:
                continue
            a_lo, a_hi = blks[0], blks[-1]
            assert blks == list(range(a_lo, a_hi + 1))
            mcol0 = len(ld_cols)
            for a in range(a_lo, a_hi + 1):
                t = 4 * st + a
                ld_cols.append(
                    ld_slots[:, slot0:slot0 + 128] - 128.0 * t)
                w_cols.append(w_slots[:, slot0:slot0 + 128])
            wins.append((ci, a_lo, a_hi - a_lo + 1, mcol0))
        win_by_group[(st, j)] = wins
    NWCOL = len(ld_cols)
    ld_stream = np.stack(ld_cols, axis=2).astype(FP16)    # [NC, 128, NWCOL]
    w_stream = np.stack(w_cols, axis=2).astype(FP16)

    # pooling: batch id per node, [128, NT] (pad -1000); per-graph recip
    batch = np.asarray(batch, dtype=np.int64)
    batch_scal = np.full((NC, 128, NT), -1000.0, dtype=np.float32)
    for k in range(NC):
        bs = batch[k * SH:(k + 1) * SH].astype(np.float32)
        pad = np.full(SHP - SH, -1000.0, dtype=np.float32)
        batch_scal[k] = np.concatenate([bs, pad]).reshape(NT, 128).T
    cnt = np.bincount(batch, minlength=G).astype(np.float32)
    recip = (1.0 / np.maximum(cnt, 1.0)).reshape(1, G)

    # AG-in DMA segments per supertile: (tile_a0, ntiles, j, rowoff)
    ag_segs = []
    for st in range(NST):
        segs = []
        a = 0
        while a < 4:
            base = 512 * st + 128 * a
            j = base // QT
            r = base - j * QT
            n = 1
            while a + n < 4 and (base + 128 * n) // QT == j:
                n += 1
            segs.append((a, n, j, r))
            a += n
        ag_segs.append(segs)
    # last supertile completing each quarter
    ag_fire_st = [((QT * (j + 1) - 1) // 512) for j in range(4)]

    meta = dict(
        N=N, D=D, SH=SH, SHP=SHP, QT=QT, NT=NT, NST=NST, TBL=TBL, G=G,
        TOT=TOT, NWCOL=NWCOL, IDXCOLS=IDXCOLS,
        groups=groups, gcol_off=gcol_off, win_by_group=win_by_group,
        ag_segs=ag_segs, ag_fire_st=ag_fire_st,
        idx_stream=idx_stream, ld_stream=ld_stream, w_stream=w_stream,
        batch_scal=batch_scal, recip=recip, dinvsq=dinvsq,
    )
    return meta


def _build(meta):
    import concourse.mybir as mybir
    import concourse.bacc as bacc
    import concourse.tile as tile

    f32 = mybir.dt.float32
    fp16 = mybir.dt.float16
    i16 = mybir.dt.int16

    SHP, QT, NT, NST, TBL, G = (meta["SHP"], meta["QT"], meta["NT"],
                                meta["NST"], meta["TBL"], meta["G"])
    NWCOL, IDXCOLS = meta["NWCOL"], meta["IDXCOLS"]
    groups, gcol_off = meta["groups"], meta["gcol_off"]
    win_by_group, ag_segs = meta["win_by_group"], meta["ag_segs"]
    ag_fire_st = meta["ag_fire_st"]
    MAXGCOL = max((g[3] // 128 for g in groups), default=1)
    MAXWIN = max((len(w) and (w[-1][3] + w[-1][2]) - w[0][3]
                  for w in win_by_group.values()), default=1)

    nc = bacc.Bacc("TRN2", target_bir_lowering=False, debug=False,
                   enable_asserts=False, num_devices=NC,
                   num_swdge_queues=4)

    # ---- I/O ----
    xT_in = nc.dram_tensor("xT_in", [128, SHP], f32, kind="ExternalInput")
    idx_in = nc.dram_tensor("idx_in", [128, IDXCOLS], i16, kind="ExternalInput")
    ld_in = nc.dram_tensor("ld_in", [128, NWCOL], fp16, kind="ExternalInput")
    w_in2 = nc.dram_tensor("w_in2", [128, NWCOL], fp16, kind="ExternalInput")
    dinvsq_in = nc.dram_tensor("dinvsq_in", [1, SHP], f32, kind="ExternalInput")
    bscal_in = nc.dram_tensor("bscal_in", [128, NT], fp16, kind="ExternalInput")
    recip_in = nc.dram_tensor("recip_in", [1, G], f32, kind="ExternalInput")
    w_in = nc.dram_tensor("w_in", [5 * 128, 128], fp16, kind="ExternalInput")
    ball_in = nc.dram_tensor("ball_in", [128, 5], f32, kind="ExternalInput")
    iota128_in = nc.dram_tensor("iota128_in", [128, 128], fp16, kind="ExternalInput")
    iotag_in = nc.dram_tensor("iotag_in", [128, G], fp16, kind="ExternalInput")
    onesr_in = nc.dram_tensor("onesr_in", [1, 128], f32, kind="ExternalInput")
    ident_in = nc.dram_tensor("ident_in", [128, 128], fp16, kind="ExternalInput")
    wl1_in = nc.dram_tensor("wl1_in", [640, 640], f32, kind="ExternalInput")
    bl1_in = nc.dram_tensor("bl1_in", [128, 5], f32, kind="ExternalInput")
    wl2_in = nc.dram_tensor("wl2_in", [128, 5], f32, kind="ExternalInput")
    bl2_in = nc.dram_tensor("bl2_in", [1, 1], f32, kind="ExternalInput")
    out_ext = nc.dram_tensor("out", [G], f32, kind="ExternalOutput")

    ag_ins, ag_outs = [], []
    for l in range(5):
        ag_ins.append([nc.dram_tensor(f"agi_{l}_{j}", [QT, 128], fp16,
                                      kind="Internal") for j in range(4)])
        ag_outs.append([nc.dram_tensor(f"ago_{l}_{j}", [TBL, 128], fp16,
                                       kind="Internal", addr_space="Shared")
                        for j in range(4)])
    ar_in = nc.dram_tensor("ar_in", [640, 512], f32, kind="Internal")
    ar_out = nc.dram_tensor("ar_out", [640, 512], f32, kind="Internal",
                            addr_space="Shared")

    AOT = mybir.AluOpType
    AFT = mybir.ActivationFunctionType

    with tile.TileContext(nc) as tc:
        with tc.tile_pool(name="const", bufs=1) as cpool, \
             tc.tile_pool(name="stream", bufs=1) as spool, \
             tc.tile_pool(name="big", bufs=1) as bpool, \
             tc.tile_pool(name="work", bufs=2) as wpool, \
             tc.tile_pool(name="tokp", bufs=6) as tokpool, \
             tc.tile_pool(name="mp", bufs=4) as mpool, \
             tc.tile_pool(name="psA", bufs=2, space="PSUM") as psA, \
             tc.tile_pool(name="psG", bufs=2, space="PSUM") as psG, \
             tc.tile_pool(name="psB", bufs=2, space="PSUM") as psB, \
             tc.tile_pool(name="psP", bufs=1, space="PSUM") as psP:

            # ---- constants / streams ----
            iota128 = cpool.tile([128, 128], fp16)
            nc.sync.dma_start(iota128[:], iota128_in.ap())
            iotag = cpool.tile([128, G], fp16)
            nc.sync.dma_start(iotag[:], iotag_in.ap())
            onesr = cpool.tile([1, 128], f32)
            nc.sync.dma_start(onesr[:], onesr_in.ap())
            ident = cpool.tile([128, 128], fp16)
            nc.sync.dma_start(ident[:], ident_in.ap())
            w_sb = cpool.tile([128, 5, 128], fp16)
            nc.sync.dma_start(w_sb[:], w_in.ap().rearrange("(a p) b -> p a b", p=128))
            ball = cpool.tile([128, 5], f32)
            nc.sync.dma_start(ball[:], ball_in.ap())
            bscal = cpool.tile([128, NT], fp16)
            nc.sync.dma_start(bscal[:], bscal_in.ap())
            idx_sb = spool.tile([128, IDXCOLS], i16)
            nc.sync.dma_start(idx_sb[:], idx_in.ap())
            ld_sb = spool.tile([128, NWCOL], fp16)
            nc.sync.dma_start(ld_sb[:], ld_in.ap())
            w_sb2 = spool.tile([128, NWCOL], fp16)
            nc.sync.dma_start(w_sb2[:], w_in2.ap())

            # dinv^2 replicated to all partitions, fp16 [128, SHP]
            dinvsq = bpool.tile([128, SHP], fp16)
            for st in range(NST):
                dvqs = wpool.tile([1, 512], f32, tag="dvqs")
                nc.sync.dma_start(dvqs[:], dinvsq_in.ap()[0:1,
                                                          512 * st:512 * st + 512])
                rps = psG.tile([128, 512], f32, tag="g")
                nc.tensor.matmul(rps[:], onesr[:], dvqs[:], start=True, stop=True)
                nc.vector.tensor_copy(dinvsq[:, 512 * st:512 * st + 512], rps[:])

            yT = [bpool.tile([128, SHP], fp16, name=f"yT{i}", tag=f"yT{i}")
                  for i in range(2)]
            nc.gpsimd.dma_start(yT[0][:], xT_in.ap())   # cast f32->fp16

            def gemm_st(l, st, src_ap):
                """u_l(st) = W_l.T-transform of src; writes u into yT[l%2]
                slice st (feat-major) and ships transposed tiles to ag_ins."""
                s0 = 512 * st
                ups = psG.tile([128, 512], f32, tag="g")
                nc.tensor.matmul(ups[:], w_sb[:, l, :], src_ap,
                                 start=True, stop=True)
                ut = yT[l % 2][:, s0:s0 + 512]
                nc.vector.tensor_copy(ut, ups[:])
                trp = psB.tile([128, 512], fp16, tag="tr")
                for a in range(4):
                    nc.tensor.transpose(trp[:, 128 * a:128 * a + 128],
                                        ut[:, 128 * a:128 * a + 128], ident[:])
                agst = wpool.tile([128, 4, 128], fp16, tag="agst")
                nc.vector.tensor_copy(
                    agst[:].rearrange("p a b -> p (a b)"), trp[:])
                for (a0, ntil, j, roff) in ag_segs[st]:
                    nc.sync.dma_start(
                        ag_ins[l][j].ap()[roff:roff + 128 * ntil, :]
                        .rearrange("(a p) b -> p a b", p=128),
                        agst[:, a0:a0 + ntil, :])

            def fire_ag(l, st):
                for j in range(4):
                    if ag_fire_st[j] == st:
                        nc.gpsimd.collective_compute(
                            "AllGather", AOT.bypass,
                            replica_groups=[list(range(NC))],
                            ins=[ag_ins[l][j].ap().opt()],
                            outs=[ag_outs[l][j].ap().opt()])

            # ---- layer 0 GEMM over all supertiles ----
            with nc.named_scope("gemm0"):
                for st in range(NST):
                    gemm_st(0, st, yT[0][:, 512 * st:512 * st + 512])
                    fire_ag(0, st)

            gq = 0  # gather queue rotation
            pool_ps = None
            for l in range(5):
                scope = nc.named_scope(f"L{l}")
                scope.__enter__()
                ycur = yT[l % 2]
                for st in range(NST):
                    s0 = 512 * st
                    zps = psA.tile([128, 512], f32, tag="z")
                    nwin = sum(len(win_by_group[(st, j)]) for j in range(4))
                    wi = 0
                    for j in range(4):
                        wins = win_by_group[(st, j)]
                        if not wins:
                            continue
                        _, _, goff, gpad = groups[st * 4 + j]
                        gcols = gpad // 128
                        tok = tokpool.tile([128, MAXGCOL, 128], fp16, tag="tok")
                        co = gcol_off[(st, j)]
                        nc.gpsimd.dma_gather(
                            tok[:, :gcols, :], ag_outs[l][j].ap(),
                            idx_sb[:, co:co + gpad // 16],
                            num_idxs=gpad, num_idxs_reg=gpad, elem_size=128,
                            single_packet=False, queue_num=gq % 4)
                        gq += 1
                        # batched M build: (iota==ld) * norm
                        m0 = wins[0][3]
                        nb = (wins[-1][3] + wins[-1][2]) - m0
                        m = mpool.tile([128, MAXWIN, 128], fp16, tag="m")
                        iota_b = iota128[:].unsqueeze(1).broadcast_to([128, nb, 128])
                        ld_b = ld_sb[:, m0:m0 + nb].unsqueeze(2) \
                            .broadcast_to([128, nb, 128])
                        w_b = w_sb2[:, m0:m0 + nb].unsqueeze(2) \
                            .broadcast_to([128, nb, 128])
                        nc.vector.tensor_tensor(m[:, :nb, :], iota_b, ld_b,
                                                AOT.is_equal)
                        nc.vector.tensor_tensor(m[:, :nb, :], m[:, :nb, :],
                                                w_b, AOT.mult)
                        for (ci, a_lo, ncells, mcol0) in wins:
                            nc.tensor.matmul(
                                zps[:, 128 * a_lo:128 * (a_lo + ncells)],
                                tok[:, ci, :],
                                m[:, mcol0 - m0:mcol0 - m0 + ncells, :],
                                start=(wi == 0), stop=(wi == nwin - 1))
                            wi += 1
                    # z += dinv^2 * u (self loop), + bias, relu
                    selft = wpool.tile([128, 512], f32, tag="selft")
                    nc.vector.tensor_tensor(selft[:], ycur[:, s0:s0 + 512],
                                            dinvsq[:, s0:s0 + 512], AOT.mult)
                    nc.vector.tensor_tensor(selft[:], zps[:], selft[:], AOT.add)
                    tmp = wpool.tile([128, 512], fp16, tag="tmp")
                    nc.scalar.activation(tmp[:], selft[:], AFT.Relu,
                                         bias=ball[:, l:l + 1])
                    # pooling of tmp: transpose -> one-hot matmuls into psP
                    trp2 = psB.tile([128, 512], fp16, tag="tr")
                    for a in range(4):
                        nc.tensor.transpose(trp2[:, 128 * a:128 * a + 128],
                                            tmp[:, 128 * a:128 * (a + 1)], ident[:])
                    ynm = wpool.tile([128, 4, 128], fp16, tag="ynm")
                    nc.vector.tensor_copy(
                        ynm[:].rearrange("p a b -> p (a b)"), trp2[:])
                    mpt = wpool.tile([128, 4, G], fp16, tag="mpt")
                    iotag_b = iotag[:].unsqueeze(1).broadcast_to([128, 4, G])
                    bs_b = bscal[:, 4 * st:4 * st + 4].unsqueeze(2) \
                        .broadcast_to([128, 4, G])
                    nc.vector.tensor_tensor(mpt[:], iotag_b, bs_b, AOT.is_equal)
                    if st == 0:
                        pool_ps = psP.tile([128, 512], f32, tag="pool")
                    for a in range(4):
                        nc.tensor.matmul(pool_ps[:, :G], ynm[:, a, :],
                                         mpt[:, a, :],
                                         start=(st == 0 and a == 0),
                                         stop=(st == NST - 1 and a == 3))
                    # next layer GEMM for this supertile
                    if l < 4:
                        gemm_st(l + 1, st, tmp[:])
                        fire_ag(l + 1, st)
                # pool partials -> ar_in rows [128l, 128(l+1))
                arst = wpool.tile([128, 512], f32, tag="arst")
                nc.vector.tensor_copy(arst[:, :G], pool_ps[:, :G])
                if G < 512:
                    nc.vector.memset(arst[:, G:], 0.0)
                nc.sync.dma_start(ar_in.ap()[128 * l:128 * (l + 1), :], arst[:])
                scope.__exit__(None, None, None)

            nc.gpsimd.collective_compute(
                "AllReduce", AOT.add, replica_groups=[list(range(NC))],
                ins=[ar_in.ap().opt()], outs=[ar_out.ap().opt()])

            # ---- MLP (replicated, fp32); scratch carved out of dead yT1 ----
            wl1 = yT[0][:, :6400].bitcast(f32).rearrange(
                "p (a b) -> p a b", a=5)
            nc.sync.dma_start(wl1,
                              wl1_in.ap().rearrange("(a p) b -> p a b", p=128))
            wl2 = cpool.tile([128, 5], f32)
            nc.sync.dma_start(wl2[:], wl2_in.ap())
            bl1 = cpool.tile([128, 5], f32)
            nc.sync.dma_start(bl1[:], bl1_in.ap())
            bl2 = cpool.tile([1, 1], f32)
            nc.sync.dma_start(bl2[:], bl2_in.ap())
            recip = cpool.tile([1, G], f32)
            nc.sync.dma_start(recip[:], recip_in.ap())

            rps = psA.tile([128, 512], f32, tag="z")
            nc.tensor.matmul(rps[:, :G], onesr[:], recip[:], start=True, stop=True)
            scratch = yT[1][:].bitcast(f32)   # [128, 6400] f32
            rrep = scratch[:, 5120:5632]
            nc.vector.tensor_copy(rrep[:, :G], rps[:, :G])

            pm = [scratch[:, 512 * t:512 * (t + 1)] for t in range(5)]
            for t in range(5):
                pt = wpool.tile([128, 512], f32, tag="pt")
                nc.sync.dma_start(pt[:], ar_out.ap()[128 * t:128 * (t + 1), :])
                nc.vector.tensor_tensor(pm[t][:, :G], pt[:, :G], rrep[:, :G],
                                        AOT.mult)
            hs = [scratch[:, 512 * (5 + o):512 * (6 + o)] for o in range(5)]
            for o in range(5):
                hps = psA.tile([128, 512], f32, tag="z")
                for i in range(5):
                    nc.tensor.matmul(hps[:, :G], wl1[:, i, 128 * o:128 * (o + 1)],
                                     pm[i][:, :G], start=(i == 0), stop=(i == 4))
                nc.scalar.activation(hs[o][:, :G], hps[:, :G], AFT.Relu,
                                     bias=bl1[:, o:o + 1])
            yps = psP.tile([1, 512], f32, tag="yf")
            for i in range(5):
                nc.tensor.matmul(yps[:, :G], wl2[:, i:i + 1], hs[i][:, :G],
                                 start=(i == 0), stop=(i == 4))
            ysb = wpool.tile([1, 512], f32, tag="ysb")
            nc.scalar.activation(ysb[:, :G], yps[:, :G], AFT.Identity,
                                 bias=bl2[:, 0:1])
            nc.sync.dma_start(out_ext.ap().rearrange("(a b) -> a b", a=1),
                              ysb[:, :G])

    nc.compile()
    return nc


def _make_in_maps(meta, x, W_list, b_list, Wl1, bl1, Wl2, bl2):
    N, D, SH, SHP, NT, G = (meta["N"], meta["D"], meta["SH"], meta["SHP"],
                            meta["NT"], meta["G"])
    iota128 = np.tile(np.arange(128), (128, 1)).astype(FP16)
    iotag = np.tile(np.arange(G), (128, 1)).astype(FP16)
    onesr = np.ones((1, 128), np.float32)
    ident = np.eye(128).astype(FP16)
    w_stack = np.concatenate([w.astype(FP16) for w in W_list], axis=0)
    ball = np.stack([b.astype(np.float32) for b in b_list], axis=1)
    bl1m = np.asarray(bl1, np.float32).reshape(5, 128).T
    wl2m = np.asarray(Wl2, np.float32).reshape(5, 128).T
    wl1m = np.asarray(Wl1, np.float32)
    bl2m = np.asarray(bl2, np.float32).reshape(1, 1)

    dinvsq = meta["dinvsq"]
    in_maps = []
    for k in range(NC):
        xs = np.asarray(x[k * SH:(k + 1) * SH], np.float32)
        xT = np.zeros((128, SHP), np.float32)
        xT[:, :SH] = xs.T
        dvq = np.zeros((1, SHP), np.float32)
        dvq[0, :SH] = dinvsq[k * SH:(k + 1) * SH]
        in_maps.append(dict(
            xT_in=xT, idx_in=meta["idx_stream"][k],
            ld_in=meta["ld_stream"][k], w_in2=meta["w_stream"][k],
            dinvsq_in=dvq, bscal_in=meta["batch_scal"][k].astype(FP16),
            recip_in=meta["recip"],
            w_in=w_stack, ball_in=ball, iota128_in=iota128, iotag_in=iotag,
            onesr_in=onesr, ident_in=ident,
            wl1_in=wl1m, bl1_in=bl1m, wl2_in=wl2m, bl2_in=bl2m,
        ))
    return in_maps


_LAST_RESULT = {}


def kernel(x, edge_index, batch, W1, b1, W2, b2, W3, b3, W4, b4,
           Wl1, bl1, Wl2, bl2, n_graphs=_G_DEFAULT, trace=False):
    from concourse import bass_utils

    x = np.asarray(x)
    meta = _preprocess(x, np.asarray(edge_index), np.asarray(batch), n_graphs)
    nc = _build(meta)
    in_maps = _make_in_maps(meta, x, [W1, W2, W3, W4, W4],
                            [b1, b2, b3, b4, b4], Wl1, bl1, Wl2, bl2)
    res = bass_utils.run_bass_kernel_spmd(
        nc, in_maps, core_ids=list(range(NC)), trace=trace)
    _LAST_RESULT["res"] = res
    return res.results[0]["out"].astype(np.float32)


# revision 5
# speedup vs baseline: 2.5931x; 1.1190x over previous
"""GCN (5x GCNConv + global_mean_pool + 2-layer MLP) on 8 Trainium2 cores.

v2 design (node-partitioned, pull-based SpMM):
  - Nodes sharded 8 ways (12500/core, padded to 12800). Per layer:
    GEMM (feat-major) -> transpose -> AllGather of the [12800,128] fp16
    shard per quarter (4 chunks, int16-indexable 25600-row tables),
    fired eagerly as soon as each quarter's GEMM tiles are written.
  - SpMM: per (supertile, quarter) group, a 4-queue-rotated dma_gather
    pulls the group's edge-source rows (self-triggered SWDGE; rotating
    queue_num engages all 4 Q7 pairs and quadruples gather throughput).
    One-hot scatter matrices M = (iota==ld)*norm are built BATCHED on
    the Vector engine with stride-0 broadcast APs (2 ops per group) and
    consumed by per-chunk merged matmuls accumulating z in PSUM.
  - Graph norms (sym-normalized adjacency incl. self-loop weights) are
    computed on the HOST and folded into M / the self-term, so there is
    no on-device degree pass.
  - Self-loops are applied as z += dinv^2 * u via a vector multiply
    (dinv^2 replicated across partitions once at start).
  - Mean-pool one-hots (0/1) are built batched per supertile; per-graph
    1/count is folded in after the AllReduce (host-provided recip).
All compute fp16 storage / fp32 PSUM accumulation; MLP fp32.
"""

import numpy as np

NC = 8
_G_DEFAULT = 512
FP16 = np.float16


def _ceil_to(a, m):
    return -(-a // m) * m


def _preprocess(x, edge_index, batch, n_graphs):
    N, D = x.shape
    assert N % NC == 0
    SH = N // NC
    SHP = _ceil_to(SH, 512)
    QT = SHP // 4
    NT = SHP // 128
    NST = SHP // 512
    TBL = NC * QT
    assert TBL < 32768
    G = n_graphs

    row = np.asarray(edge_index[0], dtype=np.int64)
    col = np.asarray(edge_index[1], dtype=np.int64)

    # host-side GCN norm: deg = in-degree + 1 (self loop), dinv = deg^-1/2
    deg = np.bincount(col, minlength=N).astype(np.float64) + 1.0
    dinv = 1.0 / np.sqrt(deg)
    dinv_f = dinv.astype(np.float32)

    kd = col // SH
    ld = col - kd * SH
    ks = row // SH
    rr = row - ks * SH
    jq = rr // QT
    idx16 = (ks * QT + (rr - jq * QT)).astype(np.int64)
    tile = ld // 128

    per_core = []
    for k in range(NC):
        m = kd == k
        o = np.lexsort((ld[m], tile[m]))
        per_core.append({
            "tile": tile[m][o], "j": jq[m][o],
            "idx16": idx16[m][o], "ld": ld[m][o],
        })

    # cells = (tile, j), padded to 16, cross-core max; >=16 for j=0 cells
    # so every supertile has at least one window (start=True coverage).
    ncell = NT * 4
    S = np.zeros(ncell, dtype=np.int64)
    for k in range(NC):
        ck = per_core[k]["tile"] * 4 + per_core[k]["j"]
        cnt = np.bincount(ck, minlength=ncell)
        S = np.maximum(S, cnt)
    S = _ceil_to(S, 16)
    S[0::4] = np.maximum(S[0::4], 16)

    # layout: groups (st, j); within a group the 4 cells (tiles a=0..3 of
    # the supertile) are contiguous; group padded to 128.
    cell_off = np.zeros(ncell, dtype=np.int64)
    groups = []            # (st, j, slot_off, padded_slots)
    off = 0
    for st in range(NST):
        for j in range(4):
            goff = off
            for a in range(4):
                c = (4 * st + a) * 4 + j
                cell_off[c] = off
                off += S[c]
            off = _ceil_to(off, 128)
            groups.append((st, j, int(goff), int(off - goff)))
    TOT = off

    # fill slot streams
    idx_slots = np.zeros((NC, TOT), dtype=np.int16)
    ld_slots = np.full((NC, TOT), -1000.0, dtype=np.float32)
    for k in range(NC):
        pk = per_core[k]
        ck = pk["tile"] * 4 + pk["j"]
        arange = np.arange(len(ck))
        if len(ck):
            order = np.argsort(ck, kind="stable")
            ck_s = ck[order]
            starts_pos = np.concatenate([[0], np.flatnonzero(np.diff(ck_s) != 0) + 1])
            first_occ = np.full(ncell, -1, dtype=np.int64)
            first_occ[ck_s[starts_pos]] = starts_pos
            within = np.empty(len(ck), dtype=np.int64)
            within[order] = arange - first_occ[ck_s]
        else:
            within = arange
        slot = cell_off[ck] + within
        idx_slots[k, slot] = pk["idx16"].astype(np.int16)
        ld_slots[k, slot] = pk["ld"].astype(np.float32)

    # idx wrap per group: [16, gpad/16] tiled to 128 partitions
    IDXCOLS = TOT // 16
    idx_stream = np.zeros((NC, 128, IDXCOLS), dtype=np.int16)
    gcol_off = {}
    coff = 0
    for (st, j, goff, gpad) in groups:
        gcol_off[(st, j)] = coff
        if gpad == 0:
            continue
        blk = idx_slots[:, goff:goff + gpad].reshape(NC, gpad // 16, 16)
        blk = np.transpose(blk, (0, 2, 1))
        idx_stream[:, :, coff:coff + gpad // 16] = np.tile(blk, (1, 8, 1))
        coff += gpad // 16

    # windows: one merged matmul per 128-slot chunk; one M column block per
    # (chunk x intersecting cell).  Blocks of a chunk are adjacent cells.
    ld_cols = []    # per block: [NC, 128] ld - 128*tile
    win_by_group = {}   # (st,j) -> list of (tok_col, a_lo, nblk, mcol0)
    for (st, j, goff, gpad) in groups:
        wins = []
        for ci in range(gpad // 128):
            slot0 = goff + ci * 128
            # cells intersecting [slot0, slot0+128)
            blks = []
            for a in range(4):
                c = (4 * st + a) * 4 + j
                c0, c1 = cell_off[c], cell_off[c] + int(S[c])
                if c0 < slot0 + 128 and c1 > slot0 and S[c] > 0:
                    blks.append(a)
            if not blks:
                continue
            a_lo, a_hi = blks[0], blks[-1]
            assert blks == list(range(a_lo, a_hi + 1))
            mcol0 = len(ld_cols)
            for a in range(a_lo, a_hi + 1):
                t = 4 * st + a
                ld_cols.append(
                    ld_slots[:, slot0:slot0 + 128] - 128.0 * t)
            wins.append((ci, a_lo, a_hi - a_lo + 1, mcol0))
        win_by_group[(st, j)] = wins
    NWCOL = len(ld_cols)
    # host-built 0/1 scatter one-hots, fp8 (exact): [NC, 128, NWCOL*128]
    import ml_dtypes
    ldc = np.stack(ld_cols, axis=2)                       # [NC, 128, NWCOL]
    m_stream = np.zeros((NC, 128, NWCOL * 128), dtype=ml_dtypes.float8_e4m3)
    car = np.arange(128, dtype=np.float32)
    for k in range(NC):
        eq = ldc[k][:, :, None] == car[None, None, :]     # [128, NWCOL, 128]
        m_stream[k] = eq.reshape(128, NWCOL * 128).astype(ml_dtypes.float8_e4m3)

    # pooling: batch id per node, [128, NT] (pad -1000); per-graph recip
    batch = np.asarray(batch, dtype=np.int64)
    batch_scal = np.full((NC, 128, NT), -1000.0, dtype=np.float32)
    for k in range(NC):
        bs = batch[k * SH:(k + 1) * SH].astype(np.float32)
        pad = np.full(SHP - SH, -1000.0, dtype=np.float32)
        batch_scal[k] = np.concatenate([bs, pad]).reshape(NT, 128).T
    cnt = np.bincount(batch, minlength=G).astype(np.float32)
    recip = (1.0 / np.maximum(cnt, 1.0)).reshape(1, G)

    # AG-in DMA segments per supertile: (tile_a0, ntiles, j, rowoff)
    ag_segs = []
    for st in range(NST):
        segs = []
        a = 0
        while a < 4:
            base = 512 * st + 128 * a
            j = base // QT
            r = base - j * QT
            n = 1
            while a + n < 4 and (base + 128 * n) // QT == j:
                n += 1
            segs.append((a, n, j, r))
            a += n
        ag_segs.append(segs)
    # last supertile completing each quarter
    ag_fire_st = [((QT * (j + 1) - 1) // 512) for j in range(4)]

    meta = dict(
        N=N, D=D, SH=SH, SHP=SHP, QT=QT, NT=NT, NST=NST, TBL=TBL, G=G,
        TOT=TOT, NWCOL=NWCOL, IDXCOLS=IDXCOLS,
        groups=groups, gcol_off=gcol_off, win_by_group=win_by_group,
        ag_segs=ag_segs, ag_fire_st=ag_fire_st,
        idx_stream=idx_stream, m_stream=m_stream,
        batch_scal=batch_scal, recip=recip, dinv=dinv_f,
    )
    return meta


def _build(meta):
    import concourse.mybir as mybir
    import concourse.bacc as bacc
    import concourse.tile as tile

    f32 = mybir.dt.float32
    fp16 = mybir.dt.float16
    fp8 = mybir.dt.float8e4
    i16 = mybir.dt.int16

    SHP, QT, NT, NST, TBL, G = (meta["SHP"], meta["QT"], meta["NT"],
                                meta["NST"], meta["TBL"], meta["G"])
    NWCOL, IDXCOLS = meta["NWCOL"], meta["IDXCOLS"]
    groups, gcol_off = meta["groups"], meta["gcol_off"]
    win_by_group, ag_segs = meta["win_by_group"], meta["ag_segs"]
    ag_fire_st = meta["ag_fire_st"]
    MAXGCOL = max((g[3] // 128 for g in groups), default=1)
    MAXWIN = max((len(w) and (w[-1][3] + w[-1][2]) - w[0][3]
                  for w in win_by_group.values()), default=1)

    nc = bacc.Bacc("TRN2", target_bir_lowering=False, debug=False,
                   enable_asserts=False, num_devices=NC,
                   num_swdge_queues=4)

    # ---- I/O ----
    xT_in = nc.dram_tensor("xT_in", [128, SHP], f32, kind="ExternalInput")
    idx_in = nc.dram_tensor("idx_in", [128, IDXCOLS], i16, kind="ExternalInput")
    m_in = nc.dram_tensor("m_in", [128, NWCOL * 128], fp8, kind="ExternalInput")
    dinv_in = nc.dram_tensor("dinv_in", [1, SHP], f32, kind="ExternalInput")
    bscal_in = nc.dram_tensor("bscal_in", [128, NT], fp16, kind="ExternalInput")
    recip_in = nc.dram_tensor("recip_in", [1, G], f32, kind="ExternalInput")
    w_in = nc.dram_tensor("w_in", [5 * 128, 128], fp16, kind="ExternalInput")
    ball_in = nc.dram_tensor("ball_in", [128, 5], f32, kind="ExternalInput")
    iota128_in = nc.dram_tensor("iota128_in", [128, 128], fp16, kind="ExternalInput")
    iotag_in = nc.dram_tensor("iotag_in", [128, G], fp16, kind="ExternalInput")
    onesr_in = nc.dram_tensor("onesr_in", [1, 128], f32, kind="ExternalInput")
    ident_in = nc.dram_tensor("ident_in", [128, 128], fp16, kind="ExternalInput")
    wl1_in = nc.dram_tensor("wl1_in", [640, 640], f32, kind="ExternalInput")
    bl1_in = nc.dram_tensor("bl1_in", [128, 5], f32, kind="ExternalInput")
    wl2_in = nc.dram_tensor("wl2_in", [128, 5], f32, kind="ExternalInput")
    bl2_in = nc.dram_tensor("bl2_in", [1, 1], f32, kind="ExternalInput")
    out_ext = nc.dram_tensor("out", [G], f32, kind="ExternalOutput")

    ag_ins, ag_outs = [], []
    for l in range(5):
        ag_ins.append([nc.dram_tensor(f"agi_{l}_{j}", [QT, 128], fp16,
                                      kind="Internal") for j in range(4)])
        ag_outs.append([nc.dram_tensor(f"ago_{l}_{j}", [TBL, 128], fp16,
                                       kind="Internal", addr_space="Shared")
                        for j in range(4)])
    ar_in = nc.dram_tensor("ar_in", [640, 512], f32, kind="Internal")
    ar_out = nc.dram_tensor("ar_out", [640, 512], f32, kind="Internal",
                            addr_space="Shared")

    AOT = mybir.AluOpType
    AFT = mybir.ActivationFunctionType

    with tile.TileContext(nc) as tc:
        with tc.tile_pool(name="const", bufs=1) as cpool, \
             tc.tile_pool(name="stream", bufs=1) as spool, \
             tc.tile_pool(name="big", bufs=1) as bpool, \
             tc.tile_pool(name="work", bufs=2) as wpool, \
             tc.tile_pool(name="tokp", bufs=6) as tokpool, \
             tc.tile_pool(name="mp", bufs=4) as mpool, \
             tc.tile_pool(name="psA", bufs=2, space="PSUM") as psA, \
             tc.tile_pool(name="psG", bufs=2, space="PSUM") as psG, \
             tc.tile_pool(name="psB", bufs=2, space="PSUM") as psB, \
             tc.tile_pool(name="psP", bufs=1, space="PSUM") as psP:

            # ---- constants / streams ----
            iota128 = cpool.tile([128, 128], fp16)
            nc.sync.dma_start(iota128[:], iota128_in.ap())
            iotag = cpool.tile([128, G], fp16)
            nc.sync.dma_start(iotag[:], iotag_in.ap())
            onesr = cpool.tile([1, 128], f32)
            nc.sync.dma_start(onesr[:], onesr_in.ap())
            ident = cpool.tile([128, 128], fp16)
            nc.sync.dma_start(ident[:], ident_in.ap())
            w_sb = cpool.tile([128, 5, 128], fp16)
            nc.sync.dma_start(w_sb[:], w_in.ap().rearrange("(a p) b -> p a b", p=128))
            ball = cpool.tile([128, 5], f32)
            nc.sync.dma_start(ball[:], ball_in.ap())
            bscal = cpool.tile([128, NT], fp16)
            nc.sync.dma_start(bscal[:], bscal_in.ap())
            idx_sb = spool.tile([128, IDXCOLS], i16)
            nc.sync.dma_start(idx_sb[:], idx_in.ap())

            # dinv replicated to all partitions, fp16 [128, SHP]
            dinv_rep = bpool.tile([128, SHP], fp16)
            for st in range(NST):
                dvqs = wpool.tile([1, 512], f32, tag="dvqs")
                nc.sync.dma_start(dvqs[:], dinv_in.ap()[0:1,
                                                        512 * st:512 * st + 512])
                rps = psG.tile([128, 512], f32, tag="g")
                nc.tensor.matmul(rps[:], onesr[:], dvqs[:], start=True, stop=True)
                nc.vector.tensor_copy(dinv_rep[:, 512 * st:512 * st + 512], rps[:])

            yT = [bpool.tile([128, SHP], fp16, name=f"yT{i}", tag=f"yT{i}")
                  for i in range(2)]
            nc.gpsimd.dma_start(yT[0][:], xT_in.ap())   # cast f32->fp16

            def gemm_st(l, st, src_ap):
                """u_l(st) = W_l.T-transform of src; writes u into yT[l%2]
                slice st (feat-major) and ships transposed tiles to ag_ins."""
                s0 = 512 * st
                ups = psG.tile([128, 512], f32, tag="g")
                nc.tensor.matmul(ups[:], w_sb[:, l, :], src_ap,
                                 start=True, stop=True)
                ut = yT[l % 2][:, s0:s0 + 512]
                nc.vector.tensor_tensor(ut, ups[:], dinv_rep[:, s0:s0 + 512],
                                        AOT.mult)
                trp = psB.tile([128, 512], fp16, tag="tr")
                for a in range(4):
                    nc.tensor.transpose(trp[:, 128 * a:128 * a + 128],
                                        ut[:, 128 * a:128 * a + 128], ident[:])
                agst = wpool.tile([128, 4, 128], fp16, tag="agst")
                nc.vector.tensor_copy(
                    agst[:].rearrange("p a b -> p (a b)"), trp[:])
                for (a0, ntil, j, roff) in ag_segs[st]:
                    nc.sync.dma_start(
                        ag_ins[l][j].ap()[roff:roff + 128 * ntil, :]
                        .rearrange("(a p) b -> p a b", p=128),
                        agst[:, a0:a0 + ntil, :])

            def fire_ag(l, st):
                for j in range(4):
                    if ag_fire_st[j] == st:
                        nc.gpsimd.collective_compute(
                            "AllGather", AOT.bypass,
                            replica_groups=[list(range(NC))],
                            ins=[ag_ins[l][j].ap().opt()],
                            outs=[ag_outs[l][j].ap().opt()])

            # ---- layer 0 GEMM over all supertiles ----
            with nc.named_scope("gemm0"):
                for st in range(NST):
                    gemm_st(0, st, yT[0][:, 512 * st:512 * st + 512])
                    fire_ag(0, st)

            gq = 0  # gather queue rotation
            pool_ps = None
            for l in range(5):
                scope = nc.named_scope(f"L{l}")
                scope.__enter__()
                ycur = yT[l % 2]
                for st in range(NST):
                    s0 = 512 * st
                    zps = psA.tile([128, 512], f32, tag="z")
                    nwin = sum(len(win_by_group[(st, j)]) for j in range(4))
                    wi = 0
                    for j in range(4):
                        wins = win_by_group[(st, j)]
                        if not wins:
                            continue
                        _, _, goff, gpad = groups[st * 4 + j]
                        gcols = gpad // 128
                        tok = tokpool.tile([128, MAXGCOL, 128], fp16, tag="tok")
                        co = gcol_off[(st, j)]
                        nc.gpsimd.dma_gather(
                            tok[:, :gcols, :], ag_outs[l][j].ap(),
                            idx_sb[:, co:co + gpad // 16],
                            num_idxs=gpad, num_idxs_reg=gpad, elem_size=128,
                            single_packet=False, queue_num=gq % 4)
                        gq += 1
                        # 0/1 scatter one-hots streamed from HBM (fp8)
                        m0 = wins[0][3]
                        nb = (wins[-1][3] + wins[-1][2]) - m0
                        m = mpool.tile([128, MAXWIN, 128], fp8, tag="m")
                        nc.sync.dma_start(
                            m[:, :nb, :],
                            m_in.ap()[:, 128 * m0:128 * (m0 + nb)]
                            .rearrange("p (a b) -> p a b", b=128))
                        for (ci, a_lo, ncells, mcol0) in wins:
                            nc.tensor.matmul(
                                zps[:, 128 * a_lo:128 * (a_lo + ncells)],
                                tok[:, ci, :],
                                m[:, mcol0 - m0:mcol0 - m0 + ncells, :],
                                start=(wi == 0), stop=(wi == nwin - 1))
                            wi += 1
                    # z = (sum_edges dinv_s u_s + dinv_d u_d) * dinv_d
                    selft = wpool.tile([128, 512], f32, tag="selft")
                    nc.vector.tensor_tensor(selft[:], zps[:],
                                            ycur[:, s0:s0 + 512], AOT.add)
                    nc.vector.tensor_tensor(selft[:], selft[:],
                                            dinv_rep[:, s0:s0 + 512], AOT.mult)
                    tmp = wpool.tile([128, 512], fp16, tag="tmp")
                    nc.scalar.activation(tmp[:], selft[:], AFT.Relu,
                                         bias=ball[:, l:l + 1])
                    # pooling of tmp: transpose -> one-hot matmuls into psP
                    trp2 = psB.tile([128, 512], fp16, tag="tr")
                    for a in range(4):
                        nc.tensor.transpose(trp2[:, 128 * a:128 * a + 128],
                                            tmp[:, 128 * a:128 * (a + 1)], ident[:])
                    ynm = wpool.tile([128, 4, 128], fp16, tag="ynm")
                    nc.vector.tensor_copy(
                        ynm[:].rearrange("p a b -> p (a b)"), trp2[:])
                    mpt = wpool.tile([128, 4, G], fp16, tag="mpt")
                    iotag_b = iotag[:].unsqueeze(1).broadcast_to([128, 4, G])
                    bs_b = bscal[:, 4 * st:4 * st + 4].unsqueeze(2) \
                        .broadcast_to([128, 4, G])
                    nc.vector.tensor_tensor(mpt[:], iotag_b, bs_b, AOT.is_equal)
                    if st == 0:
                        pool_ps = psP.tile([128, 512], f32, tag="pool")
                    for a in range(4):
                        nc.tensor.matmul(pool_ps[:, :G], ynm[:, a, :],
                                         mpt[:, a, :],
                                         start=(st == 0 and a == 0),
                                         stop=(st == NST - 1 and a == 3))
                    # next layer GEMM for this supertile
                    if l < 4:
                        gemm_st(l + 1, st, tmp[:])
                        fire_ag(l + 1, st)
                # pool partials -> ar_in rows [128l, 128(l+1))
                arst = wpool.tile([128, 512], f32, tag="arst")
                nc.vector.tensor_copy(arst[:, :G], pool_ps[:, :G])
                if G < 512:
                    nc.vector.memset(arst[:, G:], 0.0)
                nc.sync.dma_start(ar_in.ap()[128 * l:128 * (l + 1), :], arst[:])
                scope.__exit__(None, None, None)

            nc.gpsimd.collective_compute(
                "AllReduce", AOT.add, replica_groups=[list(range(NC))],
                ins=[ar_in.ap().opt()], outs=[ar_out.ap().opt()])

            # ---- MLP (replicated, fp32); scratch carved out of dead yT1 ----
            wl1 = yT[0][:, :6400].bitcast(f32).rearrange(
                "p (a b) -> p a b", a=5)
            nc.sync.dma_start(wl1,
                              wl1_in.ap().rearrange("(a p) b -> p a b", p=128))
            wl2 = cpool.tile([128, 5], f32)
            nc.sync.dma_start(wl2[:], wl2_in.ap())
            bl1 = cpool.tile([128, 5], f32)
            nc.sync.dma_start(bl1[:], bl1_in.ap())
            bl2 = cpool.tile([1, 1], f32)
            nc.sync.dma_start(bl2[:], bl2_in.ap())
            recip = cpool.tile([1, G], f32)
            nc.sync.dma_start(recip[:], recip_in.ap())

            rps = psA.tile([128, 512], f32, tag="z")
            nc.tensor.matmul(rps[:, :G], onesr[:], recip[:], start=True, stop=True)
            scratch = yT[1][:].bitcast(f32)   # [128, 6400] f32
            rrep = scratch[:, 5120:5632]
            nc.vector.tensor_copy(rrep[:, :G], rps[:, :G])

            pm = [scratch[:, 512 * t:512 * (t + 1)] for t in range(5)]
            for t in range(5):
                pt = wpool.tile([128, 512], f32, tag="pt")
                nc.sync.dma_start(pt[:], ar_out.ap()[128 * t:128 * (t + 1), :])
                nc.vector.tensor_tensor(pm[t][:, :G], pt[:, :G], rrep[:, :G],
                                        AOT.mult)
            hs = [scratch[:, 512 * (5 + o):512 * (6 + o)] for o in range(5)]
            for o in range(5):
                hps = psA.tile([128, 512], f32, tag="z")
                for i in range(5):
                    nc.tensor.matmul(hps[:, :G], wl1[:, i, 128 * o:128 * (o + 1)],
                                     pm[i][:, :G], start=(i == 0), stop=(i == 4))
                nc.scalar.activation(hs[o][:, :G], hps[:, :G], AFT.Relu,
                                     bias=bl1[:, o:o + 1])
            yps = psP.tile([1, 512], f32, tag="yf")
            for i in range(5):
                nc.tensor.matmul(yps[:, :G], wl2[:, i:i + 1], hs[i][:, :G],
                                 start=(i == 0), stop=(i == 4))
            ysb = wpool.tile([1, 512], f32, tag="ysb")
            nc.scalar.activation(ysb[:, :G], yps[:, :G], AFT.Identity,
                                 bias=bl2[:, 0:1])
            nc.sync.dma_start(out_ext.ap().rearrange("(a b) -> a b", a=1),
                              ysb[:, :G])

    nc.compile()
    return nc


def _make_in_maps(meta, x, W_list, b_list, Wl1, bl1, Wl2, bl2):
    N, D, SH, SHP, NT, G = (meta["N"], meta["D"], meta["SH"], meta["SHP"],
                            meta["NT"], meta["G"])
    iota128 = np.tile(np.arange(128), (128, 1)).astype(FP16)
    iotag = np.tile(np.arange(G), (128, 1)).astype(FP16)
    onesr = np.ones((1, 128), np.float32)
    ident = np.eye(128).astype(FP16)
    w_stack = np.concatenate([w.astype(FP16) for w in W_list], axis=0)
    ball = np.stack([b.astype(np.float32) for b in b_list], axis=1)
    bl1m = np.asarray(bl1, np.float32).reshape(5, 128).T
    wl2m = np.asarray(Wl2, np.float32).reshape(5, 128).T
    wl1m = np.asarray(Wl1, np.float32)
    bl2m = np.asarray(bl2, np.float32).reshape(1, 1)

    dinv = meta["dinv"]
    in_maps = []
    for k in range(NC):
        xs = np.asarray(x[k * SH:(k + 1) * SH], np.float32)
        xT = np.zeros((128, SHP), np.float32)
        xT[:, :SH] = xs.T
        dvq = np.zeros((1, SHP), np.float32)
        dvq[0, :SH] = dinv[k * SH:(k + 1) * SH]
        in_maps.append(dict(
            xT_in=xT, idx_in=meta["idx_stream"][k], m_in=meta["m_stream"][k],
            dinv_in=dvq, bscal_in=meta["batch_scal"][k].astype(FP16),
            recip_in=meta["recip"],
            w_in=w_stack, ball_in=ball, iota128_in=iota128, iotag_in=iotag,
            onesr_in=onesr, ident_in=ident,
            wl1_in=wl1m, bl1_in=bl1m, wl2_in=wl2m, bl2_in=bl2m,
        ))
    return in_maps


_LAST_RESULT = {}


def kernel(x, edge_index, batch, W1, b1, W2, b2, W3, b3, W4, b4,
           Wl1, bl1, Wl2, bl2, n_graphs=_G_DEFAULT, trace=False):
    from concourse import bass_utils

    x = np.asarray(x)
    meta = _preprocess(x, np.asarray(edge_index), np.asarray(batch), n_graphs)
    nc = _build(meta)
    in_maps = _make_in_maps(meta, x, [W1, W2, W3, W4, W4],
                            [b1, b2, b3, b4, b4], Wl1, bl1, Wl2, bl2)
    res = bass_utils.run_bass_kernel_spmd(
        nc, in_maps, core_ids=list(range(NC)), trace=trace)
    _LAST_RESULT["res"] = res
    return res.results[0]["out"].astype(np.float32)


# revision 6
# speedup vs baseline: 2.7561x; 1.0629x over previous
"""GCN (5x GCNConv + global_mean_pool + 2-layer MLP) on 8 Trainium2 cores.

v2 design (node-partitioned, pull-based SpMM):
  - Nodes sharded 8 ways (12500/core, padded to 12800). Per layer:
    GEMM (feat-major) -> transpose -> AllGather of the [12800,128] fp16
    shard per quarter (4 chunks, int16-indexable 25600-row tables),
    fired eagerly as soon as each quarter's GEMM tiles are written.
  - SpMM: per (supertile, quarter) group, a 4-queue-rotated dma_gather
    pulls the group's edge-source rows (self-triggered SWDGE; rotating
    queue_num engages all 4 Q7 pairs and quadruples gather throughput).
    One-hot scatter matrices M = (iota==ld)*norm are built BATCHED on
    the Vector engine with stride-0 broadcast APs (2 ops per group) and
    consumed by per-chunk merged matmuls accumulating z in PSUM.
  - Graph norms (sym-normalized adjacency incl. self-loop weights) are
    computed on the HOST and folded into M / the self-term, so there is
    no on-device degree pass.
  - Self-loops are applied as z += dinv^2 * u via a vector multiply
    (dinv^2 replicated across partitions once at start).
  - Mean-pool one-hots (0/1) are built batched per supertile; per-graph
    1/count is folded in after the AllReduce (host-provided recip).
All compute fp16 storage / fp32 PSUM accumulation; MLP fp32.
"""

import numpy as np

NC = 8
_G_DEFAULT = 512
FP16 = np.float16


def _ceil_to(a, m):
    return -(-a // m) * m


def _preprocess(x, edge_index, batch, n_graphs):
    N, D = x.shape
    assert N % NC == 0
    SH = N // NC
    SHP = _ceil_to(SH, 512)
    QT = SHP // 4
    NT = SHP // 128
    NST = SHP // 512
    TBL = NC * QT
    assert TBL < 32768
    G = n_graphs

    row = np.asarray(edge_index[0], dtype=np.int64)
    col = np.asarray(edge_index[1], dtype=np.int64)

    # host-side GCN norm: deg = in-degree + 1 (self loop), dinv = deg^-1/2
    deg = np.bincount(col, minlength=N).astype(np.float64) + 1.0
    dinv = 1.0 / np.sqrt(deg)
    dinv_f = dinv.astype(np.float32)

    kd = col // SH
    ld = col - kd * SH
    ks = row // SH
    rr = row - ks * SH
    jq = rr // QT
    idx16 = (ks * QT + (rr - jq * QT)).astype(np.int64)
    tile = ld // 128

    per_core = []
    for k in range(NC):
        m = kd == k
        o = np.lexsort((ld[m], tile[m]))
        per_core.append({
            "tile": tile[m][o], "j": jq[m][o],
            "idx16": idx16[m][o], "ld": ld[m][o],
        })

    # cells = (tile, j), padded to 16, cross-core max; >=16 for j=0 cells
    # so every supertile has at least one window (start=True coverage).
    ncell = NT * 4
    S = np.zeros(ncell, dtype=np.int64)
    for k in range(NC):
        ck = per_core[k]["tile"] * 4 + per_core[k]["j"]
        cnt = np.bincount(ck, minlength=ncell)
        S = np.maximum(S, cnt)
    S = _ceil_to(S, 16)
    S[0::4] = np.maximum(S[0::4], 16)

    # layout: groups (st, j); within a group the 4 cells (tiles a=0..3 of
    # the supertile) are contiguous; group padded to 128.
    cell_off = np.zeros(ncell, dtype=np.int64)
    groups = []            # (st, j, slot_off, padded_slots)
    off = 0
    for st in range(NST):
        for j in range(4):
            goff = off
            for a in range(4):
                c = (4 * st + a) * 4 + j
                cell_off[c] = off
                off += S[c]
            off = _ceil_to(off, 128)
            groups.append((st, j, int(goff), int(off - goff)))
    TOT = off

    # fill slot streams
    idx_slots = np.zeros((NC, TOT), dtype=np.int16)
    ld_slots = np.full((NC, TOT), -1000.0, dtype=np.float32)
    for k in range(NC):
        pk = per_core[k]
        ck = pk["tile"] * 4 + pk["j"]
        arange = np.arange(len(ck))
        if len(ck):
            order = np.argsort(ck, kind="stable")
            ck_s = ck[order]
            starts_pos = np.concatenate([[0], np.flatnonzero(np.diff(ck_s) != 0) + 1])
            first_occ = np.full(ncell, -1, dtype=np.int64)
            first_occ[ck_s[starts_pos]] = starts_pos
            within = np.empty(len(ck), dtype=np.int64)
            within[order] = arange - first_occ[ck_s]
        else:
            within = arange
        slot = cell_off[ck] + within
        idx_slots[k, slot] = pk["idx16"].astype(np.int16)
        ld_slots[k, slot] = pk["ld"].astype(np.float32)

    # idx wrap per group: [16, gpad/16] tiled to 128 partitions
    IDXCOLS = TOT // 16
    idx_stream = np.zeros((NC, 128, IDXCOLS), dtype=np.int16)
    gcol_off = {}
    coff = 0
    for (st, j, goff, gpad) in groups:
        gcol_off[(st, j)] = coff
        if gpad == 0:
            continue
        blk = idx_slots[:, goff:goff + gpad].reshape(NC, gpad // 16, 16)
        blk = np.transpose(blk, (0, 2, 1))
        idx_stream[:, :, coff:coff + gpad // 16] = np.tile(blk, (1, 8, 1))
        coff += gpad // 16

    # windows: one merged matmul per 128-slot chunk; one M column block per
    # (chunk x intersecting cell).  Blocks of a chunk are adjacent cells.
    ld_cols = []    # per block: [NC, 128] ld - 128*tile
    win_by_group = {}   # (st,j) -> list of (tok_col, a_lo, nblk, mcol0)
    for (st, j, goff, gpad) in groups:
        wins = []
        for ci in range(gpad // 128):
            slot0 = goff + ci * 128
            # cells intersecting [slot0, slot0+128)
            blks = []
            for a in range(4):
                c = (4 * st + a) * 4 + j
                c0, c1 = cell_off[c], cell_off[c] + int(S[c])
                if c0 < slot0 + 128 and c1 > slot0 and S[c] > 0:
                    blks.append(a)
            if not blks:
                continue
            a_lo, a_hi = blks[0], blks[-1]
            assert blks == list(range(a_lo, a_hi + 1))
            mcol0 = len(ld_cols)
            for a in range(a_lo, a_hi + 1):
                t = 4 * st + a
                ld_cols.append(
                    ld_slots[:, slot0:slot0 + 128] - 128.0 * t)
            wins.append((ci, a_lo, a_hi - a_lo + 1, mcol0))
        win_by_group[(st, j)] = wins
    NWCOL = len(ld_cols)
    # host-built 0/1 scatter one-hots, fp8 (exact): [NC, 128, NWCOL*128]
    import ml_dtypes
    ldc = np.stack(ld_cols, axis=2)                       # [NC, 128, NWCOL]
    m_stream = np.zeros((NC, 128, NWCOL * 128), dtype=ml_dtypes.float8_e4m3)
    car = np.arange(128, dtype=np.float32)
    for k in range(NC):
        eq = ldc[k][:, :, None] == car[None, None, :]     # [128, NWCOL, 128]
        m_stream[k] = eq.reshape(128, NWCOL * 128).astype(ml_dtypes.float8_e4m3)

    # pooling: batch id per node, [128, NT] (pad -1000); per-graph recip
    batch = np.asarray(batch, dtype=np.int64)
    batch_scal = np.full((NC, 128, NT), -1000.0, dtype=np.float32)
    for k in range(NC):
        bs = batch[k * SH:(k + 1) * SH].astype(np.float32)
        pad = np.full(SHP - SH, -1000.0, dtype=np.float32)
        batch_scal[k] = np.concatenate([bs, pad]).reshape(NT, 128).T
    cnt = np.bincount(batch, minlength=G).astype(np.float32)
    recip = (1.0 / np.maximum(cnt, 1.0)).reshape(1, G)

    # AG-in DMA segments per supertile: (tile_a0, ntiles, j, rowoff)
    ag_segs = []
    for st in range(NST):
        segs = []
        a = 0
        while a < 4:
            base = 512 * st + 128 * a
            j = base // QT
            r = base - j * QT
            n = 1
            while a + n < 4 and (base + 128 * n) // QT == j:
                n += 1
            segs.append((a, n, j, r))
            a += n
        ag_segs.append(segs)
    # last supertile completing each quarter
    ag_fire_st = [((QT * (j + 1) - 1) // 512) for j in range(4)]

    meta = dict(
        N=N, D=D, SH=SH, SHP=SHP, QT=QT, NT=NT, NST=NST, TBL=TBL, G=G,
        TOT=TOT, NWCOL=NWCOL, IDXCOLS=IDXCOLS,
        groups=groups, gcol_off=gcol_off, win_by_group=win_by_group,
        ag_segs=ag_segs, ag_fire_st=ag_fire_st,
        idx_stream=idx_stream, m_stream=m_stream,
        batch_scal=batch_scal, recip=recip, dinv=dinv_f,
    )
    return meta


def _build(meta):
    import concourse.mybir as mybir
    import concourse.bacc as bacc
    import concourse.tile as tile

    f32 = mybir.dt.float32
    fp16 = mybir.dt.float16
    fp8 = mybir.dt.float8e4
    i16 = mybir.dt.int16

    SHP, QT, NT, NST, TBL, G = (meta["SHP"], meta["QT"], meta["NT"],
                                meta["NST"], meta["TBL"], meta["G"])
    NWCOL, IDXCOLS = meta["NWCOL"], meta["IDXCOLS"]
    groups, gcol_off = meta["groups"], meta["gcol_off"]
    win_by_group, ag_segs = meta["win_by_group"], meta["ag_segs"]
    ag_fire_st = meta["ag_fire_st"]
    MAXGCOL = max((g[3] // 128 for g in groups), default=1)
    MAXWIN = max((len(w) and (w[-1][3] + w[-1][2]) - w[0][3]
                  for w in win_by_group.values()), default=1)

    nc = bacc.Bacc("TRN2", target_bir_lowering=False, debug=False,
                   enable_asserts=False, num_devices=NC,
                   num_swdge_queues=4)

    # ---- I/O ----
    xT_in = nc.dram_tensor("xT_in", [128, SHP], f32, kind="ExternalInput")
    idx_in = nc.dram_tensor("idx_in", [128, IDXCOLS], i16, kind="ExternalInput")
    m_in = nc.dram_tensor("m_in", [128, NWCOL * 128], fp8, kind="ExternalInput")
    dinv_in = nc.dram_tensor("dinv_in", [1, SHP], f32, kind="ExternalInput")
    bscal_in = nc.dram_tensor("bscal_in", [128, NT], fp16, kind="ExternalInput")
    recip_in = nc.dram_tensor("recip_in", [1, G], f32, kind="ExternalInput")
    w_in = nc.dram_tensor("w_in", [5 * 128, 128], fp16, kind="ExternalInput")
    ball_in = nc.dram_tensor("ball_in", [128, 5], f32, kind="ExternalInput")
    iota128_in = nc.dram_tensor("iota128_in", [128, 128], fp16, kind="ExternalInput")
    iotag_in = nc.dram_tensor("iotag_in", [128, G], fp16, kind="ExternalInput")
    onesr_in = nc.dram_tensor("onesr_in", [1, 128], f32, kind="ExternalInput")
    ident_in = nc.dram_tensor("ident_in", [128, 128], fp16, kind="ExternalInput")
    wl1_in = nc.dram_tensor("wl1_in", [640, 640], f32, kind="ExternalInput")
    bl1_in = nc.dram_tensor("bl1_in", [128, 5], f32, kind="ExternalInput")
    wl2_in = nc.dram_tensor("wl2_in", [128, 5], f32, kind="ExternalInput")
    bl2_in = nc.dram_tensor("bl2_in", [1, 1], f32, kind="ExternalInput")
    out_ext = nc.dram_tensor("out", [G], f32, kind="ExternalOutput")

    ag_ins, ag_outs = [], []
    for l in range(5):
        ag_ins.append([nc.dram_tensor(f"agi_{l}_{j}", [QT, 128], fp16,
                                      kind="Internal") for j in range(4)])
        ag_outs.append([nc.dram_tensor(f"ago_{l}_{j}", [TBL, 128], fp16,
                                       kind="Internal", addr_space="Shared")
                        for j in range(4)])
    ar_in = nc.dram_tensor("ar_in", [640, 512], f32, kind="Internal")
    ar_out = nc.dram_tensor("ar_out", [640, 512], f32, kind="Internal",
                            addr_space="Shared")

    AOT = mybir.AluOpType
    AFT = mybir.ActivationFunctionType

    with tile.TileContext(nc) as tc:
        with tc.tile_pool(name="const", bufs=1) as cpool, \
             tc.tile_pool(name="stream", bufs=1) as spool, \
             tc.tile_pool(name="big", bufs=1) as bpool, \
             tc.tile_pool(name="work", bufs=2) as wpool, \
             tc.tile_pool(name="tokp", bufs=8) as tokpool, \
             tc.tile_pool(name="mp", bufs=6) as mpool, \
             tc.tile_pool(name="psA", bufs=2, space="PSUM") as psA, \
             tc.tile_pool(name="psG", bufs=2, space="PSUM") as psG, \
             tc.tile_pool(name="psB", bufs=2, space="PSUM") as psB, \
             tc.tile_pool(name="psP", bufs=1, space="PSUM") as psP:

            # ---- constants / streams ----
            iota128 = cpool.tile([128, 128], fp16)
            nc.sync.dma_start(iota128[:], iota128_in.ap())
            iotag = cpool.tile([128, G], fp16)
            nc.sync.dma_start(iotag[:], iotag_in.ap())
            onesr = cpool.tile([1, 128], f32)
            nc.sync.dma_start(onesr[:], onesr_in.ap())
            ident = cpool.tile([128, 128], fp16)
            nc.sync.dma_start(ident[:], ident_in.ap())
            w_sb = cpool.tile([128, 5, 128], fp16)
            nc.sync.dma_start(w_sb[:], w_in.ap().rearrange("(a p) b -> p a b", p=128))
            ball = cpool.tile([128, 5], f32)
            nc.sync.dma_start(ball[:], ball_in.ap())
            bscal = cpool.tile([128, NT], fp16)
            nc.sync.dma_start(bscal[:], bscal_in.ap())
            idx_sb = spool.tile([128, IDXCOLS], i16)
            nc.sync.dma_start(idx_sb[:], idx_in.ap())

            # dinv replicated to all partitions, fp16 [128, SHP]
            dinv_rep = bpool.tile([128, SHP], fp16)
            for st in range(NST):
                dvqs = wpool.tile([1, 512], f32, tag="dvqs")
                nc.sync.dma_start(dvqs[:], dinv_in.ap()[0:1,
                                                        512 * st:512 * st + 512])
                rps = psG.tile([128, 512], f32, tag="g")
                nc.tensor.matmul(rps[:], onesr[:], dvqs[:], start=True, stop=True)
                nc.vector.tensor_copy(dinv_rep[:, 512 * st:512 * st + 512], rps[:])

            yT = [bpool.tile([128, SHP], fp16, name=f"yT{i}", tag=f"yT{i}")
                  for i in range(2)]
            nc.gpsimd.dma_start(yT[0][:], xT_in.ap())   # cast f32->fp16

            def gemm_st(l, st, src_ap):
                """u_l(st) = W_l.T-transform of src; writes u into yT[l%2]
                slice st (feat-major) and ships transposed tiles to ag_ins."""
                s0 = 512 * st
                ups = psG.tile([128, 512], f32, tag="g")
                nc.tensor.matmul(ups[:], w_sb[:, l, :], src_ap,
                                 start=True, stop=True)
                ut = yT[l % 2][:, s0:s0 + 512]
                nc.vector.tensor_tensor(ut, ups[:], dinv_rep[:, s0:s0 + 512],
                                        AOT.mult)
                trp = psB.tile([128, 512], fp16, tag="tr")
                for a in range(4):
                    nc.tensor.transpose(trp[:, 128 * a:128 * a + 128],
                                        ut[:, 128 * a:128 * a + 128], ident[:])
                agst = wpool.tile([128, 4, 128], fp16, tag="agst")
                nc.vector.tensor_copy(
                    agst[:].rearrange("p a b -> p (a b)"), trp[:])
                for (a0, ntil, j, roff) in ag_segs[st]:
                    nc.sync.dma_start(
                        ag_ins[l][j].ap()[roff:roff + 128 * ntil, :]
                        .rearrange("(a p) b -> p a b", p=128),
                        agst[:, a0:a0 + ntil, :])

            def fire_ag(l, st):
                for j in range(4):
                    if ag_fire_st[j] == st:
                        nc.gpsimd.collective_compute(
                            "AllGather", AOT.bypass,
                            replica_groups=[list(range(NC))],
                            ins=[ag_ins[l][j].ap().opt()],
                            outs=[ag_outs[l][j].ap().opt()])

            # ---- layer 0 GEMM over all supertiles ----
            with nc.named_scope("gemm0"):
                for st in range(NST):
                    gemm_st(0, st, yT[0][:, 512 * st:512 * st + 512])
                    fire_ag(0, st)

            gq = 0  # gather queue rotation
            pool_ps = None
            for l in range(5):
                scope = nc.named_scope(f"L{l}")
                scope.__enter__()
                ycur = yT[l % 2]
                for st in range(NST):
                    s0 = 512 * st
                    zps = psA.tile([128, 512], f32, tag="z")
                    nwin = sum(len(win_by_group[(st, j)]) for j in range(4))
                    wi = 0
                    for j in range(4):
                        wins = win_by_group[(st, j)]
                        if not wins:
                            continue
                        _, _, goff, gpad = groups[st * 4 + j]
                        gcols = gpad // 128
                        tok = tokpool.tile([128, MAXGCOL, 128], fp16, tag="tok")
                        co = gcol_off[(st, j)]
                        nc.gpsimd.dma_gather(
                            tok[:, :gcols, :], ag_outs[l][j].ap(),
                            idx_sb[:, co:co + gpad // 16],
                            num_idxs=gpad, num_idxs_reg=gpad, elem_size=128,
                            single_packet=False, queue_num=gq % 4)
                        gq += 1
                        # 0/1 scatter one-hots streamed from HBM (fp8)
                        m0 = wins[0][3]
                        nb = (wins[-1][3] + wins[-1][2]) - m0
                        m = mpool.tile([128, MAXWIN, 128], fp8, tag="m")
                        nc.scalar.dma_start(
                            m[:, :nb, :],
                            m_in.ap()[:, 128 * m0:128 * (m0 + nb)]
                            .rearrange("p (a b) -> p a b", b=128))
                        for (ci, a_lo, ncells, mcol0) in wins:
                            nc.tensor.matmul(
                                zps[:, 128 * a_lo:128 * (a_lo + ncells)],
                                tok[:, ci, :],
                                m[:, mcol0 - m0:mcol0 - m0 + ncells, :],
                                start=(wi == 0), stop=(wi == nwin - 1))
                            wi += 1
                    # z = (sum_edges dinv_s u_s + dinv_d u_d) * dinv_d
                    selft = wpool.tile([128, 512], f32, tag="selft")
                    nc.vector.tensor_tensor(selft[:], zps[:],
                                            ycur[:, s0:s0 + 512], AOT.add)
                    nc.vector.tensor_tensor(selft[:], selft[:],
                                            dinv_rep[:, s0:s0 + 512], AOT.mult)
                    tmp = wpool.tile([128, 512], fp16, tag="tmp")
                    nc.scalar.activation(tmp[:], selft[:], AFT.Relu,
                                         bias=ball[:, l:l + 1])
                    # pooling of tmp: transpose -> one-hot matmuls into psP
                    trp2 = psB.tile([128, 512], fp16, tag="tr")
                    for a in range(4):
                        nc.tensor.transpose(trp2[:, 128 * a:128 * a + 128],
                                            tmp[:, 128 * a:128 * (a + 1)], ident[:])
                    ynm = wpool.tile([128, 4, 128], fp16, tag="ynm")
                    nc.vector.tensor_copy(
                        ynm[:].rearrange("p a b -> p (a b)"), trp2[:])
                    mpt = wpool.tile([128, 4, G], fp16, tag="mpt")
                    iotag_b = iotag[:].unsqueeze(1).broadcast_to([128, 4, G])
                    bs_b = bscal[:, 4 * st:4 * st + 4].unsqueeze(2) \
                        .broadcast_to([128, 4, G])
                    nc.vector.tensor_tensor(mpt[:], iotag_b, bs_b, AOT.is_equal)
                    if st == 0:
                        pool_ps = psP.tile([128, 512], f32, tag="pool")
                    for a in range(4):
                        nc.tensor.matmul(pool_ps[:, :G], ynm[:, a, :],
                                         mpt[:, a, :],
                                         start=(st == 0 and a == 0),
                                         stop=(st == NST - 1 and a == 3))
                    # next layer GEMM for this supertile
                    if l < 4:
                        gemm_st(l + 1, st, tmp[:])
                        fire_ag(l + 1, st)
                # pool partials -> ar_in rows [128l, 128(l+1))
                arst = wpool.tile([128, 512], f32, tag="arst")
                nc.vector.tensor_copy(arst[:, :G], pool_ps[:, :G])
                if G < 512:
                    nc.vector.memset(arst[:, G:], 0.0)
                nc.sync.dma_start(ar_in.ap()[128 * l:128 * (l + 1), :], arst[:])
                scope.__exit__(None, None, None)

            nc.gpsimd.collective_compute(
                "AllReduce", AOT.add, replica_groups=[list(range(NC))],
                ins=[ar_in.ap().opt()], outs=[ar_out.ap().opt()])

            # ---- MLP (replicated, fp32); scratch carved out of dead yT1 ----
            wl1 = yT[0][:, :6400].bitcast(f32).rearrange(
                "p (a b) -> p a b", a=5)
            nc.sync.dma_start(wl1,
                              wl1_in.ap().rearrange("(a p) b -> p a b", p=128))
            wl2 = cpool.tile([128, 5], f32)
            nc.sync.dma_start(wl2[:], wl2_in.ap())
            bl1 = cpool.tile([128, 5], f32)
            nc.sync.dma_start(bl1[:], bl1_in.ap())
            bl2 = cpool.tile([1, 1], f32)
            nc.sync.dma_start(bl2[:], bl2_in.ap())
            recip = cpool.tile([1, G], f32)
            nc.sync.dma_start(recip[:], recip_in.ap())

            rps = psA.tile([128, 512], f32, tag="z")
            nc.tensor.matmul(rps[:, :G], onesr[:], recip[:], start=True, stop=True)
            scratch = yT[1][:].bitcast(f32)   # [128, 6400] f32
            rrep = scratch[:, 5120:5632]
            nc.vector.tensor_copy(rrep[:, :G], rps[:, :G])

            pm = [scratch[:, 512 * t:512 * (t + 1)] for t in range(5)]
            for t in range(5):
                pt = wpool.tile([128, 512], f32, tag="pt")
                nc.sync.dma_start(pt[:], ar_out.ap()[128 * t:128 * (t + 1), :])
                nc.vector.tensor_tensor(pm[t][:, :G], pt[:, :G], rrep[:, :G],
                                        AOT.mult)
            hs = [scratch[:, 512 * (5 + o):512 * (6 + o)] for o in range(5)]
            for o in range(5):
                hps = psA.tile([128, 512], f32, tag="z")
                for i in range(5):
                    nc.tensor.matmul(hps[:, :G], wl1[:, i, 128 * o:128 * (o + 1)],
                                     pm[i][:, :G], start=(i == 0), stop=(i == 4))
                nc.scalar.activation(hs[o][:, :G], hps[:, :G], AFT.Relu,
                                     bias=bl1[:, o:o + 1])
            yps = psP.tile([1, 512], f32, tag="yf")
            for i in range(5):
                nc.tensor.matmul(yps[:, :G], wl2[:, i:i + 1], hs[i][:, :G],
                                 start=(i == 0), stop=(i == 4))
            ysb = wpool.tile([1, 512], f32, tag="ysb")
            nc.scalar.activation(ysb[:, :G], yps[:, :G], AFT.Identity,
                                 bias=bl2[:, 0:1])
            nc.sync.dma_start(out_ext.ap().rearrange("(a b) -> a b", a=1),
                              ysb[:, :G])

    nc.compile()
    return nc


def _make_in_maps(meta, x, W_list, b_list, Wl1, bl1, Wl2, bl2):
    N, D, SH, SHP, NT, G = (meta["N"], meta["D"], meta["SH"], meta["SHP"],
                            meta["NT"], meta["G"])
    iota128 = np.tile(np.arange(128), (128, 1)).astype(FP16)
    iotag = np.tile(np.arange(G), (128, 1)).astype(FP16)
    onesr = np.ones((1, 128), np.float32)
    ident = np.eye(128).astype(FP16)
    w_stack = np.concatenate([w.astype(FP16) for w in W_list], axis=0)
    ball = np.stack([b.astype(np.float32) for b in b_list], axis=1)
    bl1m = np.asarray(bl1, np.float32).reshape(5, 128).T
    wl2m = np.asarray(Wl2, np.float32).reshape(5, 128).T
    wl1m = np.asarray(Wl1, np.float32)
    bl2m = np.asarray(bl2, np.float32).reshape(1, 1)

    dinv = meta["dinv"]
    in_maps = []
    for k in range(NC):
        xs = np.asarray(x[k * SH:(k + 1) * SH], np.float32)
        xT = np.zeros((128, SHP), np.float32)
        xT[:, :SH] = xs.T
        dvq = np.zeros((1, SHP), np.float32)
        dvq[0, :SH] = dinv[k * SH:(k + 1) * SH]
        in_maps.append(dict(
            xT_in=xT, idx_in=meta["idx_stream"][k], m_in=meta["m_stream"][k],
            dinv_in=dvq, bscal_in=meta["batch_scal"][k].astype(FP16),
            recip_in=meta["recip"],
            w_in=w_stack, ball_in=ball, iota128_in=iota128, iotag_in=iotag,
            onesr_in=onesr, ident_in=ident,
            wl1_in=wl1m, bl1_in=bl1m, wl2_in=wl2m, bl2_in=bl2m,
        ))
    return in_maps


_LAST_RESULT = {}


def kernel(x, edge_index, batch, W1, b1, W2, b2, W3, b3, W4, b4,
           Wl1, bl1, Wl2, bl2, n_graphs=_G_DEFAULT, trace=False):
    from concourse import bass_utils

    x = np.asarray(x)
    meta = _preprocess(x, np.asarray(edge_index), np.asarray(batch), n_graphs)
    nc = _build(meta)
    in_maps = _make_in_maps(meta, x, [W1, W2, W3, W4, W4],
                            [b1, b2, b3, b4, b4], Wl1, bl1, Wl2, bl2)
    res = bass_utils.run_bass_kernel_spmd(
        nc, in_maps, core_ids=list(range(NC)), trace=trace)
    _LAST_RESULT["res"] = res
    return res.results[0]["out"].astype(np.float32)
